# revision 24
# baseline (speedup 1.0000x reference)
"""YOLO loss (nms_detection) Trainium2 Bass kernel.

Data parallel over 8 NeuronCores (4 images per core). Per (image, layer):
  - y_true is host-augmented with per-cell (gx, gy, aw, ah) -> 89 channels,
    so one TensorEngine gather fetches labels + grid + anchors together.
  - cells are paired two-per-partition-row so every DMA descriptor moves
    >= 680B contiguous (full HBM rate).
  - decode pred boxes (sigmoid via exp+reciprocal; one ACT table set).
  - obj compaction: row cumsum (tensor_tensor_scan) + triangular-matmul
    partition offsets -> rank; one-hot S = (iota == rank*obj).
  - gather true-box rows via fp32r matmuls (256-wide two-block rhs).
  - broadcast box quantities via DRAM-roundtrip DMA.
  - IoU ignore mask in fp16: big [128, S, M] broadcast-AP DVE ops testing
    3*inter >= a1+a2 (equiv. IoU >= 0.5, no division).
  - dense conf BCE on c = sigmoid(x) with weight max(obj, ignore)*valid;
    obj-masked xy/wh/cls losses on the gathered [M, 174] rows only.
"""

from contextlib import ExitStack

import numpy as np

ANCHORS = np.array([[116., 90.], [156., 198.], [373., 326.],
                    [30., 61.], [62., 45.], [59., 119.],
                    [10., 13.], [16., 30.], [33., 23.]], dtype=np.float32)
IMG_W = 416.0
P = 128
B_CORE = 4
N_CORES = 8
YW = 89           # augmented y_true row: 85 + (gx, gy, aw, ah)
PW = 85
RW = 4 * YW + 4 * PW   # 696: [ytA..ytD | predA..predD] (fp16, quad cells)
TAILPAD = 48      # gather rhs reads up to row_base + 696 + 43 -> pad 48
SW = 174          # gathered sparse row: yt_aug 89 + pred 85
PADV = -60.0

# per-layer: N cells, slots S (=ceil(N/128) padded even), grid W, offsets
LAYERS = [
    dict(N=507,  S=4,  W=13.0, coff=0,    goff=0),
    dict(N=2028, S=16, W=26.0, coff=507,  goff=4),
    dict(N=8112, S=64, W=52.0, coff=2535, goff=20),
]
STOT = 84

_NC_CACHE = {}


def _make_consts():
    # dense grid/anchor const: (gxw, gyw, awhalf, ahhalf, valid)
    gad = np.zeros((P, STOT, 5), np.float32)
    # per-cell ga columns appended to y_true, in flat cell order
    percell = np.zeros((10647, 4), np.float32)
    for li, lay in enumerate(LAYERS):
        W = int(lay["W"])
        N, S, goff, coff = lay["N"], lay["S"], lay["goff"], lay["coff"]
        c = np.arange(N)
        percell[coff:coff + N, 0] = (c % (W * 3)) // 3
        percell[coff:coff + N, 1] = c // (W * 3)
        percell[coff:coff + N, 2] = ANCHORS[3 * li + (c % 3), 0]
        percell[coff:coff + N, 3] = ANCHORS[3 * li + (c % 3), 1]
        p = np.arange(P)[:, None]
        s = np.arange(S)[None, :]
        cell = (s // 4) * 512 + 4 * p + (s % 4)
        valid = cell < N
        cc = np.minimum(cell, N - 1)
        aw = ANCHORS[3 * li + (cc % 3), 0]
        ah = ANCHORS[3 * li + (cc % 3), 1]
        gx = ((cc % (W * 3)) // 3).astype(np.float32)
        gy = (cc // (W * 3)).astype(np.float32)
        gad[:, goff:goff + S, 0] = np.where(valid, gx / W, 0)
        gad[:, goff:goff + S, 1] = np.where(valid, gy / W, 0)
        gad[:, goff:goff + S, 2] = np.where(valid, aw / (2.0 * W), 0)
        gad[:, goff:goff + S, 3] = np.where(valid, ah / (2.0 * W), 0)
        gad[:, goff:goff + S, 4] = valid.astype(np.float32)
    ut = np.triu(np.ones((P, P), np.float32), 1)  # ut[q,p]=1 iff q<p
    ones128 = np.ones((P, 1), np.float32)
    sel = np.zeros((P, B_CORE), np.float32)
    for i in range(B_CORE):
        sel[32 * i:32 * (i + 1), i] = 1.0
    return {"gad": gad, "ut": ut, "ones128": ones128, "sel": sel}, percell


def build_nc(Ms):
    import concourse.bass as bass
    import concourse.bacc as bacc
    import concourse.mybir as mybir
    from concourse.tile import TileContext

    F32 = mybir.dt.float32
    F16 = mybir.dt.float16
    F32R = mybir.dt.float32r
    ALU = mybir.AluOpType
    ACT = mybir.ActivationFunctionType
    AX = mybir.AxisListType
    MM = max(Ms)

    nc = bacc.Bacc()
    yt_d = nc.dram_tensor("yt", [B_CORE, 10647, YW], F16,
                          kind="ExternalInput")
    pr_d = [nc.dram_tensor(f"p{i}", [B_CORE, LAYERS[i]["N"], PW], F16,
                           kind="ExternalInput") for i in range(3)]
    pf_d = nc.dram_tensor("pf", [B_CORE, 10647, 5], F32,
                          kind="ExternalInput")
    ga_d = nc.dram_tensor("gad", [P, STOT, 5], F32, kind="ExternalInput")
    ut_d = nc.dram_tensor("ut", [P, P], F32, kind="ExternalInput")
    on_d = nc.dram_tensor("ones128", [P, 1], F32, kind="ExternalInput")
    se_d = nc.dram_tensor("sel", [P, B_CORE], F32, kind="ExternalInput")
    loss_d = nc.dram_tensor("loss", [B_CORE, 1], F32, kind="ExternalOutput")

    def bmid(ap2, n):
        # [P, X] -> [P, n, X] (step-0 middle dim)
        return bass.AP(tensor=ap2.tensor, offset=ap2.offset,
                       ap=[ap2.ap[0], [0, n]] + ap2.ap[1:])

    big = MM > 32   # fallback config must fit SBUF with M=64
    with TileContext(nc) as tc, ExitStack() as ctx:
        cpool = ctx.enter_context(tc.tile_pool(name="consts", bufs=1))
        combp = {li: ctx.enter_context(
            tc.tile_pool(name=f"comb{li}",
                         bufs=1 if (big and li == 2) else 2))
                 for li in range(3)}
        decp = ctx.enter_context(tc.tile_pool(name="dec", bufs=3))
        ioup = ctx.enter_context(tc.tile_pool(name="iou", bufs=1))
        stp = ctx.enter_context(tc.tile_pool(name="st", bufs=2 if big else 3))
        gatp = ctx.enter_context(
            tc.tile_pool(name="gat", bufs=2 if big else 3))
        spap = ctx.enter_context(tc.tile_pool(name="spa", bufs=1))
        accp = ctx.enter_context(tc.tile_pool(name="acc", bufs=1))
        drp = ctx.enter_context(
            tc.tile_pool(name="scr", bufs=3, space=bass.MemorySpace.DRAM))
        psg = ctx.enter_context(
            tc.tile_pool(name="psg", bufs=3, space=bass.MemorySpace.PSUM))
        pso = ctx.enter_context(
            tc.tile_pool(name="pso", bufs=2, space=bass.MemorySpace.PSUM))

        GAD = cpool.tile([P, STOT, 5], F32)
        nc.sync.dma_start(out=GAD, in_=ga_d[:])
        UT = cpool.tile([P, P], F32)
        nc.sync.dma_start(out=UT, in_=ut_d[:])
        ON128 = cpool.tile([P, 1], F32)
        nc.sync.dma_start(out=ON128, in_=on_d[:])
        SELC = cpool.tile([P, B_CORE], F32)
        nc.sync.dma_start(out=SELC, in_=se_d[:])
        IOTA = cpool.tile([P, MM], F32)
        nc.gpsimd.iota(IOTA[:], [[1, MM]], base=1, channel_multiplier=0,
                       allow_small_or_imprecise_dtypes=True)
        ZER = cpool.tile([P, 64], F32)
        nc.gpsimd.memset(ZER[:], 0.0)

        ACCD = accp.tile([P, B_CORE * 9], F32)   # (img, layer, term) dense
        SACC = accp.tile([P, 9], F32)            # (layer, term) sparse
        nc.gpsimd.memset(SACC[:], 0.0)
        SPA = {li: spap.tile([P, SW], F32, tag=f"spa{li}", name=f"spa{li}")
               for li in range(3)}
        for li in range(3):
            nc.gpsimd.memset(SPA[li][:], 0.0)

        for img in range(B_CORE):
            for li in (2, 1, 0):
                lay = LAYERS[li]
                N, S, W, coff, goff = (lay["N"], lay["S"], lay["W"],
                                       lay["coff"], lay["goff"])
                M = Ms[li]
                Gp = S // 4                # quad rows
                full = N // 512            # full quad rows
                remc = N - full * 512
                rem_p = remc // 4
                odd = remc % 4             # 0..3 extra cells on one partition
                CF = combp[li].tile([P, Gp * RW + TAILPAD], F16,
                                    tag=f"comb{li}", name=f"comb{li}_{img}")
                cfl = CF[:]
                pstride = cfl.ap[0]

                def yv(c0, c1, _a=cfl, _g=Gp):
                    # yt view [P, Gp, 4, c1-c0]
                    return bass.AP(tensor=_a.tensor, offset=_a.offset + c0,
                                   ap=[_a.ap[0], [RW, _g], [YW, 4],
                                       [1, c1 - c0]])

                def pv(c0, c1, _a=cfl, _g=Gp):
                    return bass.AP(tensor=_a.tensor,
                                   offset=_a.offset + 4 * YW + c0,
                                   ap=[_a.ap[0], [RW, _g], [PW, 4],
                                       [1, c1 - c0]])

                def cview(off, n, _a=cfl):
                    return bass.AP(tensor=_a.tensor, offset=_a.offset + off,
                                   ap=[_a.ap[0], [1, n]])

                # pad init: tail cols + last quad row (dma overwrites live)
                nc.vector.memset(cview(Gp * RW, TAILPAD), 0.0)
                if remc:
                    nc.vector.memset(cview((Gp - 1) * RW, 4 * YW), 0.0)
                    nc.vector.memset(cview((Gp - 1) * RW + 4 * YW, 4 * PW),
                                     PADV)
                # ---- loads (contiguous >= 680B elements) ----
                ysrc = yt_d[img]
                ybase = ysrc.offset + coff * YW
                if full:
                    nc.sync.dma_start(
                        out=bass.AP(tensor=cfl.tensor, offset=cfl.offset,
                                    ap=[[pstride[0], P], [RW, full],
                                        [1, 4 * YW]]),
                        in_=bass.AP(tensor=ysrc.tensor, offset=ybase,
                                    ap=[[4 * YW, P], [512 * YW, full],
                                        [1, 4 * YW]]))
                if rem_p:
                    nc.sync.dma_start(
                        out=bass.AP(tensor=cfl.tensor,
                                    offset=cfl.offset + full * RW,
                                    ap=[[pstride[0], rem_p], [1, 4 * YW]]),
                        in_=bass.AP(tensor=ysrc.tensor,
                                    offset=ybase + full * 512 * YW,
                                    ap=[[4 * YW, rem_p], [1, 4 * YW]]))
                if odd:
                    nc.sync.dma_start(
                        out=CF[rem_p:rem_p + 1,
                               full * RW:full * RW + odd * YW],
                        in_=bass.AP(
                            tensor=ysrc.tensor,
                            offset=ybase + (full * 512 + 4 * rem_p) * YW,
                            ap=[[odd * YW, 1], [1, odd * YW]]))
                psrc = pr_d[li][img]
                pbase = psrc.offset
                if full:
                    nc.sync.dma_start(
                        out=bass.AP(tensor=cfl.tensor,
                                    offset=cfl.offset + 4 * YW,
                                    ap=[[pstride[0], P], [RW, full],
                                        [1, 4 * PW]]),
                        in_=bass.AP(tensor=psrc.tensor, offset=pbase,
                                    ap=[[4 * PW, P], [512 * PW, full],
                                        [1, 4 * PW]]))
                if rem_p:
                    nc.sync.dma_start(
                        out=bass.AP(tensor=cfl.tensor,
                                    offset=cfl.offset + full * RW + 4 * YW,
                                    ap=[[pstride[0], rem_p], [1, 4 * PW]]),
                        in_=bass.AP(tensor=psrc.tensor,
                                    offset=pbase + full * 512 * PW,
                                    ap=[[4 * PW, rem_p], [1, 4 * PW]]))
                if odd:
                    nc.sync.dma_start(
                        out=CF[rem_p:rem_p + 1,
                               full * RW + 4 * YW:full * RW + 4 * YW +
                               odd * PW],
                        in_=bass.AP(
                            tensor=psrc.tensor,
                            offset=pbase + (full * 512 + 4 * rem_p) * PW,
                            ap=[[odd * PW, 1], [1, odd * PW]]))

                # fp32 front pred channels (conf, xy, wh) for dense decode
                FW = 20  # 4 cells x 5 ch
                PF = decp.tile([P, Gp * FW + 20], F32, tag="pf",
                               name=f"pf{li}_{img}")
                pfl = PF[:]
                pfsrc = pf_d[img]
                pfbase = pfsrc.offset + coff * 5
                if remc:
                    nc.vector.memset(
                        bass.AP(tensor=pfl.tensor,
                                offset=pfl.offset + (Gp - 1) * FW,
                                ap=[pfl.ap[0], [1, FW]]), PADV)
                nc.vector.memset(
                    bass.AP(tensor=pfl.tensor, offset=pfl.offset + Gp * FW,
                            ap=[pfl.ap[0], [1, 20]]), PADV)
                if full:
                    nc.sync.dma_start(
                        out=bass.AP(tensor=pfl.tensor, offset=pfl.offset,
                                    ap=[[pfl.ap[0][0], P], [FW, full],
                                        [1, FW]]),
                        in_=bass.AP(tensor=pfsrc.tensor, offset=pfbase,
                                    ap=[[FW, P], [512 * 5, full], [1, FW]]))
                if rem_p:
                    nc.sync.dma_start(
                        out=bass.AP(tensor=pfl.tensor,
                                    offset=pfl.offset + full * FW,
                                    ap=[[pfl.ap[0][0], rem_p], [1, FW]]),
                        in_=bass.AP(tensor=pfsrc.tensor,
                                    offset=pfbase + full * 512 * 5,
                                    ap=[[FW, rem_p], [1, FW]]))
                if odd:
                    nc.sync.dma_start(
                        out=PF[rem_p:rem_p + 1,
                               full * FW:full * FW + odd * 5],
                        in_=bass.AP(
                            tensor=pfsrc.tensor,
                            offset=pfbase + (full * 512 + 4 * rem_p) * 5,
                            ap=[[odd * 5, 1], [1, odd * 5]]))

                def pfv(c0, c1, _a=pfl, _g=Gp):
                    return bass.AP(tensor=_a.tensor, offset=_a.offset + c0,
                                   ap=[_a.ap[0], [FW, _g], [5, 4],
                                       [1, c1 - c0]])

                # compact copies of the interleaved dense channels
                OBJC = decp.tile([P, S], F32, tag="objc")
                oc = OBJC[:]
                nc.vector.tensor_copy(
                    bass.AP(tensor=oc.tensor, offset=oc.offset,
                            ap=[oc.ap[0], [4, Gp], [1, 4]]),
                    yv(0, 1).squeeze(3))
                XCF = decp.tile([P, S], F32, tag="xcf")
                xc = XCF[:]
                nc.scalar.copy(
                    bass.AP(tensor=xc.tensor, offset=xc.offset,
                            ap=[xc.ap[0], [4, Gp], [1, 4]]),
                    pfv(0, 1).squeeze(3))

                def compact2(tile):   # [P, Gp, 4, 2] view over [P, S, 2]
                    a = tile[:]
                    return bass.AP(tensor=a.tensor, offset=a.offset,
                                   ap=[a.ap[0], [8, Gp], [2, 4], [1, 2]])

                # ---- decode dense ----
                EXY = decp.tile([P, S, 2], F32, tag="exy")
                nc.scalar.activation(compact2(EXY), pfv(1, 3), ACT.Exp,
                                     scale=-1.0)
                nc.vector.tensor_scalar_add(EXY[:], EXY[:], 1.0)
                SGX = decp.tile([P, S, 2], F32, tag="sgx")
                nc.vector.reciprocal(SGX[:], EXY[:])
                CXY = decp.tile([P, S, 2], F32, tag="cxy")
                nc.vector.scalar_tensor_tensor(
                    CXY[:], SGX[:], 1.0 / W, GAD[:, goff:goff + S, 0:2],
                    ALU.mult, ALU.add)
                EWH = decp.tile([P, S, 2], F32, tag="ewh")
                nc.scalar.activation(compact2(EWH), pfv(3, 5), ACT.Exp)
                HWT = decp.tile([P, S, 2], F32, tag="hwt")
                nc.vector.tensor_mul(HWT[:], EWH[:],
                                     GAD[:, goff:goff + S, 2:4])
                PMX = decp.tile([P, S, 2], F16, tag="pmx")
                nc.vector.tensor_add(PMX[:], CXY[:], HWT[:])
                PMN = decp.tile([P, S, 2], F16, tag="pmn")
                nc.vector.tensor_sub(PMN[:], CXY[:], HWT[:])
                A13 = decp.tile([P, S], F16, tag="a13")
                nc.vector.scalar_tensor_tensor(
                    A13[:], HWT[:, :, 0], 4.0 / 3.0, HWT[:, :, 1],
                    ALU.mult, ALU.mult)

                # ---- rank & one-hot selection ----
                RCUM = decp.tile([P, S], F32, tag="rcum")
                nc.vector.tensor_tensor_scan(RCUM[:], OBJC[:], ZER[:, 0:S],
                                             0.0, ALU.add, ALU.add)
                OFFP = pso.tile([P, 1], F32, tag="offp")
                nc.tensor.matmul(OFFP[:], UT[:], RCUM[:, S - 1:S],
                                 start=True, stop=True)
                RANK = decp.tile([P, S], F32, tag="rank")
                nc.vector.tensor_scalar_add(RANK[:], RCUM[:], OFFP[:])
                RPM = decp.tile([P, S], F32, tag="rpm")
                nc.vector.tensor_mul(RPM[:], RANK[:], OBJC[:])
                STT = stp.tile([P, S, M], F16, tag="st")
                nc.vector.tensor_tensor(STT[:], bmid(IOTA[:, 0:M], S),
                                        RPM[:].broadcast_to([P, S, M]),
                                        ALU.is_equal)

                # ---- gather true rows (PE, fp16, 256-wide 2-block rhs) ----
                PGA = psg.tile([MM, 256], F32, tag="pga")
                for s in range(S):
                    g, j = s // 4, s % 4
                    yoff = g * RW + j * YW
                    delta = 4 * YW + j * PW - j * YW  # 356 - 4*j
                    rhs = bass.AP(tensor=cfl.tensor,
                                  offset=cfl.offset + yoff,
                                  ap=[[pstride[0], P], [delta, 2], [1, 128]])
                    nc.tensor.matmul(PGA[0:M, :], STT[:, s, :],
                                     rhs, start=(s == 0), stop=(s == S - 1))
                SPT = gatp.tile([MM, SW], F32, tag="spt")
                nc.scalar.copy(SPT[0:M, 0:YW], PGA[0:M, 0:YW])
                nc.scalar.copy(SPT[0:M, YW:SW], PGA[0:M, 128:128 + PW])
                nc.sync.dma_start(out=SPA[li][32 * img:32 * img + M, :],
                                  in_=SPT[0:M, :])
                # box rows (obj,x,y,w,h) -> dram -> [P,5,M] broadcast
                SCR = drp.tile([5, MM], F32, tag="scr")
                s1 = SPT[0:M, 0:5]
                s1t = bass.AP(tensor=s1.tensor, offset=s1.offset,
                              ap=[s1.ap[0], [1, 5], [1, 1]])
                d1 = bass.AP(tensor=SCR[:].tensor, offset=SCR[:].offset,
                             ap=[[1, M], [MM, 5], [1, 1]])
                nc.sync.dma_start(out=d1, in_=s1t)
                RAWB = gatp.tile([P, 5, MM], F32, tag="rawb")
                s2 = bass.AP(tensor=SCR[:].tensor, offset=SCR[:].offset,
                             ap=[[0, P], [1, 5 * MM]])
                d2 = bass.AP(tensor=RAWB[:].tensor, offset=RAWB[:].offset,
                             ap=[RAWB[:].ap[0], [1, 5 * MM]])
                nc.sync.dma_start(out=d2, in_=s2)

                BT = gatp.tile([P, 5, MM], F16, tag="bt")
                X_, Y_, W_, H_ = (RAWB[:, 1, 0:M], RAWB[:, 2, 0:M],
                                  RAWB[:, 3, 0:M], RAWB[:, 4, 0:M])
                nc.vector.scalar_tensor_tensor(BT[:, 0, 0:M], W_, -0.5, X_,
                                               ALU.mult, ALU.add)
                nc.vector.scalar_tensor_tensor(BT[:, 1, 0:M], W_, 0.5, X_,
                                               ALU.mult, ALU.add)
                nc.vector.scalar_tensor_tensor(BT[:, 2, 0:M], H_, -0.5, Y_,
                                               ALU.mult, ALU.add)
                nc.vector.scalar_tensor_tensor(BT[:, 3, 0:M], H_, 0.5, Y_,
                                               ALU.mult, ALU.add)
                nc.vector.scalar_tensor_tensor(BT[:, 4, 0:M], W_, 1.0 / 3.0,
                                               H_, ALU.mult, ALU.mult)

                # ---- IoU ignore: smax = max_m(rx*ry - a1/3 - a2/3) ----
                shp = [P, S, M]
                IX = ioup.tile(shp, F16, tag="ix")
                nc.vector.tensor_tensor(IX[:], PMX[:, :, 0].broadcast_to(shp),
                                        bmid(BT[:, 1, 0:M], S), ALU.min)
                JX = ioup.tile(shp, F16, tag="jx")
                nc.vector.tensor_tensor(JX[:], PMN[:, :, 0].broadcast_to(shp),
                                        bmid(BT[:, 0, 0:M], S), ALU.max)
                nc.vector.tensor_sub(IX[:], IX[:], JX[:])
                nc.vector.tensor_scalar_max(IX[:], IX[:], 0.0)
                IY = ioup.tile(shp, F16, tag="iy")
                nc.vector.tensor_tensor(IY[:], PMX[:, :, 1].broadcast_to(shp),
                                        bmid(BT[:, 3, 0:M], S), ALU.min)
                JY = ioup.tile(shp, F16, tag="jy")
                nc.vector.tensor_tensor(JY[:], PMN[:, :, 1].broadcast_to(shp),
                                        bmid(BT[:, 2, 0:M], S), ALU.max)
                nc.vector.tensor_sub(IY[:], IY[:], JY[:])
                nc.vector.tensor_scalar_max(IY[:], IY[:], 0.0)
                nc.vector.tensor_mul(JX[:], IX[:], IY[:])
                nc.vector.tensor_tensor(JX[:], JX[:],
                                        A13[:].broadcast_to(shp),
                                        ALU.subtract)
                nc.vector.tensor_tensor(JX[:], JX[:], bmid(BT[:, 4, 0:M], S),
                                        ALU.subtract)
                SMX = decp.tile([P, S], F32, tag="smx")
                nc.vector.tensor_reduce(SMX[:], JX[:], axis=AX.X, op=ALU.max)

                # ---- dense conf loss (on c = sigmoid(x)) ----
                WT = decp.tile([P, S], F32, tag="wt")
                nc.vector.scalar_tensor_tensor(WT[:], SMX[:], 0.0, OBJC[:],
                                               ALU.is_lt, ALU.max)
                nc.vector.tensor_mul(WT[:], WT[:], GAD[:, goff:goff + S, 4])
                ECF = decp.tile([P, S], F32, tag="ecf")
                nc.scalar.activation(ECF[:], XCF[:], ACT.Exp, scale=-1.0)
                nc.vector.tensor_scalar_add(ECF[:], ECF[:], 1.0)
                CCF = decp.tile([P, S], F32, tag="ccf")
                nc.vector.reciprocal(CCF[:], ECF[:])
                E3T = decp.tile([P, S], F32, tag="e3t")
                nc.scalar.activation(E3T[:], CCF[:], ACT.Exp, scale=-1.0)
                L1T = decp.tile([P, S], F32, tag="l1t")
                nc.scalar.activation(L1T[:], E3T[:], ACT.Ln, bias=1.0)
                SCRP = decp.tile([P, S], F32, tag="scrp")
                base = img * 9 + li * 3
                nc.vector.scalar_tensor_tensor(
                    SCRP[:], CCF[:], 1.0, WT[:], ALU.mult, ALU.mult,
                    accum_out=ACCD[:, base:base + 1])
                nc.vector.scalar_tensor_tensor(
                    SCRP[:], L1T[:], 1.0, WT[:], ALU.mult, ALU.mult,
                    accum_out=ACCD[:, base + 1:base + 2])
                nc.vector.scalar_tensor_tensor(
                    SCRP[:], CCF[:], 1.0, OBJC[:], ALU.mult, ALU.mult,
                    accum_out=ACCD[:, base + 2:base + 3])

        # ---- sparse losses per layer (4 images batched on partitions) ----
        for li, lay in enumerate(LAYERS):
            W = lay["W"]
            Sp = SPA[li]
            obj = Sp[:, 0:1]
            WH1 = spap.tile([P, 1], F32, tag="wh1")
            nc.vector.tensor_mul(WH1[:], Sp[:, 3:4], Sp[:, 4:5])
            SC = spap.tile([P, 1], F32, tag="sc")
            nc.vector.tensor_scalar(SC[:], WH1[:], -1.0, 2.0, ALU.mult,
                                    ALU.add)
            OSC = spap.tile([P, 1], F32, tag="osc")
            nc.vector.tensor_mul(OSC[:], SC[:], obj)
            IV = spap.tile([P, 1], F32, tag="iv")
            nc.vector.tensor_scalar(IV[:], obj, -1.0, 1.0, ALU.mult, ALU.add)
            # xy
            EX = spap.tile([P, 2], F32, tag="ex")
            nc.scalar.activation(EX[:], Sp[:, 90:92], ACT.Exp, scale=-1.0)
            nc.vector.tensor_scalar_add(EX[:], EX[:], 1.0)
            SG = spap.tile([P, 2], F32, tag="sg")
            nc.vector.reciprocal(SG[:], EX[:])
            CX = spap.tile([P, 2], F32, tag="cx")
            nc.vector.tensor_add(CX[:], SG[:], Sp[:, 85:87])
            nc.vector.tensor_scalar_mul(CX[:], CX[:], 1.0 / W)
            TX = spap.tile([P, 2], F32, tag="tx")
            nc.vector.scalar_tensor_tensor(TX[:], Sp[:, 1:3], W, Sp[:, 85:87],
                                           ALU.mult, ALU.subtract)
            EB = spap.tile([P, 2], F32, tag="eb")
            nc.scalar.activation(EB[:], CX[:], ACT.Exp, scale=-1.0)
            LB = spap.tile([P, 2], F32, tag="lb")
            nc.scalar.activation(LB[:], EB[:], ACT.Ln, bias=1.0)
            OMT = spap.tile([P, 2], F32, tag="omt")
            nc.vector.tensor_scalar(OMT[:], TX[:], -1.0, 1.0, ALU.mult,
                                    ALU.add)
            VV = spap.tile([P, 2], F32, tag="vv")
            nc.vector.tensor_mul(VV[:], OMT[:], CX[:])
            nc.vector.tensor_add(VV[:], VV[:], LB[:])
            SCR2 = spap.tile([P, 2], F32, tag="scr2")
            nc.vector.tensor_scalar(SCR2[:], VV[:], OSC[:], 0.0, ALU.mult,
                                    ALU.add,
                                    accum_out=SACC[:, 3 * li:3 * li + 1])
            # wh
            T1 = spap.tile([P, 2], F32, tag="t1")
            nc.vector.tensor_scalar(T1[:], Sp[:, 3:5], IMG_W, IV[:], ALU.mult,
                                    ALU.add)
            nc.scalar.activation(T1[:], T1[:], ACT.Ln)
            T2 = spap.tile([P, 2], F32, tag="t2")
            nc.vector.tensor_scalar_add(T2[:], Sp[:, 87:89], IV[:])
            nc.scalar.activation(T2[:], T2[:], ACT.Ln)
            nc.vector.tensor_sub(T1[:], T1[:], T2[:])   # true_wh
            EW2 = spap.tile([P, 2], F32, tag="ew2")
            nc.scalar.activation(EW2[:], Sp[:, 92:94], ACT.Exp)
            AN = spap.tile([P, 2], F32, tag="an")
            nc.vector.tensor_scalar_mul(AN[:], Sp[:, 87:89], 1.0 / W)
            nc.vector.tensor_mul(EW2[:], EW2[:], AN[:])  # pred wh
            nc.vector.tensor_sub(T1[:], T1[:], EW2[:])
            DW2 = spap.tile([P, 2], F32, tag="dw2")
            nc.scalar.activation(DW2[:], T1[:], ACT.Square)
            OSC5 = spap.tile([P, 1], F32, tag="osc5")
            nc.vector.tensor_scalar_mul(OSC5[:], OSC[:], 0.5)
            nc.vector.tensor_scalar(SCR2[:], DW2[:], OSC5[:], 0.0, ALU.mult,
                                    ALU.add,
                                    accum_out=SACC[:, 3 * li + 1:3 * li + 2])
            # cls
            EC = spap.tile([P, 80], F32, tag="ec")
            nc.scalar.activation(EC[:], Sp[:, 94:174], ACT.Exp, scale=-1.0)
            nc.vector.tensor_scalar_add(EC[:], EC[:], 1.0)
            SGC = spap.tile([P, 80], F32, tag="sgc")
            nc.vector.reciprocal(SGC[:], EC[:])
            EB2 = spap.tile([P, 80], F32, tag="eb2")
            nc.scalar.activation(EB2[:], SGC[:], ACT.Exp, scale=-1.0)
            LB2 = spap.tile([P, 80], F32, tag="lb2")
            nc.scalar.activation(LB2[:], EB2[:], ACT.Ln, bias=1.0)
            OM2 = spap.tile([P, 80], F32, tag="om2")
            nc.vector.tensor_scalar(OM2[:], Sp[:, 5:85], -1.0, 1.0, ALU.mult,
                                    ALU.add)
            nc.vector.tensor_mul(OM2[:], OM2[:], SGC[:])
            nc.vector.tensor_add(OM2[:], OM2[:], LB2[:])
            SCR3 = spap.tile([P, 80], F32, tag="scr3")
            nc.vector.tensor_scalar(SCR3[:], OM2[:], obj, 0.0, ALU.mult,
                                    ALU.add,
                                    accum_out=SACC[:, 3 * li + 2:3 * li + 3])

        # ---- final combine ----
        AC3 = ACCD[:].rearrange("p (x t) -> p x t", t=3)
        TMP = accp.tile([P, B_CORE * 3], F32)
        nc.vector.tensor_add(TMP[:], AC3[:, :, 0], AC3[:, :, 1])
        nc.vector.tensor_tensor(TMP[:], TMP[:], AC3[:, :, 2], ALU.subtract)
        FIN = accp.tile([P, B_CORE], F32)
        nc.vector.tensor_reduce(
            FIN[:], TMP[:].rearrange("p (i l) -> p i l", l=3),
            axis=AX.X, op=ALU.add)
        FSP = accp.tile([P, 1], F32)
        nc.vector.tensor_reduce(FSP[:], SACC[:], axis=AX.X, op=ALU.add)
        PL = pso.tile([B_CORE, 1], F32, tag="pl")
        nc.tensor.matmul(PL[:], FIN[:], ON128[:], start=True, stop=False)
        nc.tensor.matmul(PL[:], SELC[:], FSP[:], start=False, stop=True)
        OUT = accp.tile([B_CORE, 1], F32)
        nc.scalar.copy(OUT[:], PL[:])
        nc.sync.dma_start(out=loss_d[:], in_=OUT[:])

    nc.finalize()
    return nc


def _prep_core_inputs(y_true, pred_13, pred_26, pred_52):
    consts, percell = _make_consts()
    yt85 = np.asarray(y_true).reshape(32, 10647, 85)
    yt = np.empty((32, 10647, YW), np.float16)
    yt[:, :, 0:85] = yt85
    yt[:, :, 85:89] = percell[None]
    ps32 = [np.asarray(p).reshape(32, -1, 85)
            for p in (pred_13, pred_26, pred_52)]
    ps = [np.ascontiguousarray(p.astype(np.float16)) for p in ps32]
    pf = np.ascontiguousarray(
        np.concatenate([p[:, :, 0:5] for p in ps32], axis=1))
    in_maps = []
    for c in range(N_CORES):
        sl = slice(c * B_CORE, (c + 1) * B_CORE)
        m = {"yt": yt[sl], "p0": ps[0][sl], "p1": ps[1][sl],
             "p2": ps[2][sl], "pf": pf[sl]}
        m.update(consts)
        in_maps.append(m)
    return in_maps


def kernel(y_true, pred_13, pred_26, pred_52):
    from concourse.bass_utils import run_bass_kernel_spmd

    Ms = [8, 16, 32]
    obj = np.asarray(y_true)[..., 0].reshape(32, 10647)
    cnt = [obj[:, LAYERS[i]["coff"]:LAYERS[i]["coff"] + LAYERS[i]["N"]]
           .sum(1).max() for i in range(3)]
    if any(cnt[i] > Ms[i] for i in range(3)):
        Ms = [64, 64, 64]
    key = tuple(Ms)
    if key not in _NC_CACHE:
        _NC_CACHE[key] = build_nc(Ms)
    nc = _NC_CACHE[key]

    in_maps = _prep_core_inputs(y_true, pred_13, pred_26, pred_52)
    res = run_bass_kernel_spmd(nc, in_maps, core_ids=list(range(N_CORES)))
    out = np.concatenate([r["loss"].reshape(B_CORE) for r in res.results])
    return out.astype(np.float32)


# revision 26
# speedup vs baseline: 1.0642x; 1.0642x over previous
"""YOLO loss (nms_detection) Trainium2 Bass kernel.

Data parallel over 8 NeuronCores (4 images per core). Per (image, layer):
  - y_true is host-augmented with per-cell (gx, gy, aw, ah) -> 89 channels,
    so one TensorEngine gather fetches labels + grid + anchors together.
  - inputs are host-cast to fp16 and cells quad-packed per partition row
    so every DMA descriptor moves >= 680B contiguous at half the bytes.
  - decode pred boxes (sigmoid via exp+reciprocal; one ACT table set).
  - obj compaction: row cumsum (tensor_tensor_scan) + triangular-matmul
    partition offsets -> rank; one-hot S = (iota == rank*obj).
  - gather true-box rows via fp16 matmuls (256-wide two-block rhs);
    dense decode reads a small fp32 copy of the conf/xy/wh channels.
  - broadcast box quantities via DRAM-roundtrip DMA.
  - IoU ignore mask in fp16: big [128, S, M] broadcast-AP DVE ops testing
    3*inter >= a1+a2 (equiv. IoU >= 0.5, no division).
  - dense conf BCE on c = sigmoid(x) with weight max(obj, ignore)*valid;
    obj-masked xy/wh/cls losses on the gathered [M, 174] rows only.
"""

from contextlib import ExitStack

import numpy as np

ANCHORS = np.array([[116., 90.], [156., 198.], [373., 326.],
                    [30., 61.], [62., 45.], [59., 119.],
                    [10., 13.], [16., 30.], [33., 23.]], dtype=np.float32)
IMG_W = 416.0
P = 128
B_CORE = 4
N_CORES = 8
YW = 89           # augmented y_true row: 85 + (gx, gy, aw, ah)
PW = 85
RW = 4 * YW + 4 * PW   # 696: [ytA..ytD | predA..predD] (fp16, quad cells)
TAILPAD = 48      # gather rhs reads up to row_base + 696 + 43 -> pad 48
SW = 174          # gathered sparse row: yt_aug 89 + pred 85
PADV = -60.0

# per-layer: N cells, slots S (=ceil(N/128) padded even), grid W, offsets
LAYERS = [
    dict(N=507,  S=4,  W=13.0, coff=0,    goff=0),
    dict(N=2028, S=16, W=26.0, coff=507,  goff=4),
    dict(N=8112, S=64, W=52.0, coff=2535, goff=20),
]
STOT = 84

_NC_CACHE = {}


def _make_consts():
    # dense grid/anchor const: (gxw, gyw, awhalf, ahhalf, valid)
    gad = np.zeros((P, STOT, 5), np.float32)
    # per-cell ga columns appended to y_true, in flat cell order
    percell = np.zeros((10647, 4), np.float32)
    for li, lay in enumerate(LAYERS):
        W = int(lay["W"])
        N, S, goff, coff = lay["N"], lay["S"], lay["goff"], lay["coff"]
        c = np.arange(N)
        percell[coff:coff + N, 0] = (c % (W * 3)) // 3
        percell[coff:coff + N, 1] = c // (W * 3)
        percell[coff:coff + N, 2] = ANCHORS[3 * li + (c % 3), 0]
        percell[coff:coff + N, 3] = ANCHORS[3 * li + (c % 3), 1]
        p = np.arange(P)[:, None]
        s = np.arange(S)[None, :]
        cell = (s // 4) * 512 + 4 * p + (s % 4)
        valid = cell < N
        cc = np.minimum(cell, N - 1)
        aw = ANCHORS[3 * li + (cc % 3), 0]
        ah = ANCHORS[3 * li + (cc % 3), 1]
        gx = ((cc % (W * 3)) // 3).astype(np.float32)
        gy = (cc // (W * 3)).astype(np.float32)
        gad[:, goff:goff + S, 0] = np.where(valid, gx / W, 0)
        gad[:, goff:goff + S, 1] = np.where(valid, gy / W, 0)
        gad[:, goff:goff + S, 2] = np.where(valid, aw / (2.0 * W), 0)
        gad[:, goff:goff + S, 3] = np.where(valid, ah / (2.0 * W), 0)
        gad[:, goff:goff + S, 4] = valid.astype(np.float32)
    ut = np.triu(np.ones((P, P), np.float32), 1)  # ut[q,p]=1 iff q<p
    ones128 = np.ones((P, 1), np.float32)
    sel = np.zeros((P, B_CORE), np.float32)
    for i in range(B_CORE):
        sel[32 * i:32 * (i + 1), i] = 1.0
    return {"gad": gad, "ut": ut, "ones128": ones128, "sel": sel}, percell


def build_nc(Ms):
    import concourse.bass as bass
    import concourse.bacc as bacc
    import concourse.mybir as mybir
    from concourse.tile import TileContext

    F32 = mybir.dt.float32
    F16 = mybir.dt.float16
    F32R = mybir.dt.float32r
    ALU = mybir.AluOpType
    ACT = mybir.ActivationFunctionType
    AX = mybir.AxisListType
    MM = max(Ms)

    nc = bacc.Bacc()
    yt_d = nc.dram_tensor("yt", [B_CORE, 10647, YW], F16,
                          kind="ExternalInput")
    pr_d = [nc.dram_tensor(f"p{i}", [B_CORE, LAYERS[i]["N"], PW], F16,
                           kind="ExternalInput") for i in range(3)]
    pf_d = nc.dram_tensor("pf", [B_CORE, 10647, 5], F32,
                          kind="ExternalInput")
    ga_d = nc.dram_tensor("gad", [P, STOT, 5], F32, kind="ExternalInput")
    ut_d = nc.dram_tensor("ut", [P, P], F32, kind="ExternalInput")
    on_d = nc.dram_tensor("ones128", [P, 1], F32, kind="ExternalInput")
    se_d = nc.dram_tensor("sel", [P, B_CORE], F32, kind="ExternalInput")
    loss_d = nc.dram_tensor("loss", [B_CORE, 1], F32, kind="ExternalOutput")

    def bmid(ap2, n):
        # [P, X] -> [P, n, X] (step-0 middle dim)
        return bass.AP(tensor=ap2.tensor, offset=ap2.offset,
                       ap=[ap2.ap[0], [0, n]] + ap2.ap[1:])

    big = MM > 32   # fallback config must fit SBUF with M=64
    with TileContext(nc) as tc, ExitStack() as ctx:
        cpool = ctx.enter_context(tc.tile_pool(name="consts", bufs=1))
        combp = {li: ctx.enter_context(
            tc.tile_pool(name=f"comb{li}",
                         bufs=1 if (big and li == 2) else 3))
                 for li in range(3)}
        decp = ctx.enter_context(tc.tile_pool(name="dec", bufs=2 if big else 4))
        ioup = ctx.enter_context(tc.tile_pool(name="iou", bufs=1 if big else 2))
        stp = ctx.enter_context(tc.tile_pool(name="st", bufs=2 if big else 4))
        gatp = ctx.enter_context(
            tc.tile_pool(name="gat", bufs=2 if big else 3))
        spap = ctx.enter_context(tc.tile_pool(name="spa", bufs=1))
        accp = ctx.enter_context(tc.tile_pool(name="acc", bufs=1))
        drp = ctx.enter_context(
            tc.tile_pool(name="scr", bufs=3, space=bass.MemorySpace.DRAM))
        psg = ctx.enter_context(
            tc.tile_pool(name="psg", bufs=3, space=bass.MemorySpace.PSUM))
        pso = ctx.enter_context(
            tc.tile_pool(name="pso", bufs=2, space=bass.MemorySpace.PSUM))

        GAD = cpool.tile([P, STOT, 5], F32)
        nc.sync.dma_start(out=GAD, in_=ga_d[:])
        UT = cpool.tile([P, P], F32)
        nc.sync.dma_start(out=UT, in_=ut_d[:])
        ON128 = cpool.tile([P, 1], F32)
        nc.sync.dma_start(out=ON128, in_=on_d[:])
        SELC = cpool.tile([P, B_CORE], F32)
        nc.sync.dma_start(out=SELC, in_=se_d[:])
        IOTA = cpool.tile([P, MM], F32)
        nc.gpsimd.iota(IOTA[:], [[1, MM]], base=1, channel_multiplier=0,
                       allow_small_or_imprecise_dtypes=True)
        ZER = cpool.tile([P, 64], F32)
        nc.gpsimd.memset(ZER[:], 0.0)

        ACCD = accp.tile([P, B_CORE * 9], F32)   # (img, layer, term) dense
        SACC = accp.tile([P, 9], F32)            # (layer, term) sparse
        nc.gpsimd.memset(SACC[:], 0.0)
        SPA = {li: spap.tile([P, SW], F32, tag=f"spa{li}", name=f"spa{li}")
               for li in range(3)}
        for li in range(3):
            nc.gpsimd.memset(SPA[li][:], 0.0)

        for img in range(B_CORE):
            for li in (2, 1, 0):
                lay = LAYERS[li]
                N, S, W, coff, goff = (lay["N"], lay["S"], lay["W"],
                                       lay["coff"], lay["goff"])
                M = Ms[li]
                Gp = S // 4                # quad rows
                full = N // 512            # full quad rows
                remc = N - full * 512
                rem_p = remc // 4
                odd = remc % 4             # 0..3 extra cells on one partition
                CF = combp[li].tile([P, Gp * RW + TAILPAD], F16,
                                    tag=f"comb{li}", name=f"comb{li}_{img}")
                cfl = CF[:]
                pstride = cfl.ap[0]

                def yv(c0, c1, _a=cfl, _g=Gp):
                    # yt view [P, Gp, 4, c1-c0]
                    return bass.AP(tensor=_a.tensor, offset=_a.offset + c0,
                                   ap=[_a.ap[0], [RW, _g], [YW, 4],
                                       [1, c1 - c0]])

                def pv(c0, c1, _a=cfl, _g=Gp):
                    return bass.AP(tensor=_a.tensor,
                                   offset=_a.offset + 4 * YW + c0,
                                   ap=[_a.ap[0], [RW, _g], [PW, 4],
                                       [1, c1 - c0]])

                def cview(off, n, _a=cfl):
                    return bass.AP(tensor=_a.tensor, offset=_a.offset + off,
                                   ap=[_a.ap[0], [1, n]])

                # pad init: tail cols + last quad row (dma overwrites live)
                nc.vector.memset(cview(Gp * RW, TAILPAD), 0.0)
                if remc:
                    nc.vector.memset(cview((Gp - 1) * RW, 4 * YW), 0.0)
                    nc.vector.memset(cview((Gp - 1) * RW + 4 * YW, 4 * PW),
                                     PADV)
                # ---- loads (contiguous >= 680B elements) ----
                ysrc = yt_d[img]
                ybase = ysrc.offset + coff * YW
                if full:
                    nc.sync.dma_start(
                        out=bass.AP(tensor=cfl.tensor, offset=cfl.offset,
                                    ap=[[pstride[0], P], [RW, full],
                                        [1, 4 * YW]]),
                        in_=bass.AP(tensor=ysrc.tensor, offset=ybase,
                                    ap=[[4 * YW, P], [512 * YW, full],
                                        [1, 4 * YW]]))
                if rem_p:
                    nc.sync.dma_start(
                        out=bass.AP(tensor=cfl.tensor,
                                    offset=cfl.offset + full * RW,
                                    ap=[[pstride[0], rem_p], [1, 4 * YW]]),
                        in_=bass.AP(tensor=ysrc.tensor,
                                    offset=ybase + full * 512 * YW,
                                    ap=[[4 * YW, rem_p], [1, 4 * YW]]))
                if odd:
                    nc.sync.dma_start(
                        out=CF[rem_p:rem_p + 1,
                               full * RW:full * RW + odd * YW],
                        in_=bass.AP(
                            tensor=ysrc.tensor,
                            offset=ybase + (full * 512 + 4 * rem_p) * YW,
                            ap=[[odd * YW, 1], [1, odd * YW]]))
                psrc = pr_d[li][img]
                pbase = psrc.offset
                if full:
                    nc.sync.dma_start(
                        out=bass.AP(tensor=cfl.tensor,
                                    offset=cfl.offset + 4 * YW,
                                    ap=[[pstride[0], P], [RW, full],
                                        [1, 4 * PW]]),
                        in_=bass.AP(tensor=psrc.tensor, offset=pbase,
                                    ap=[[4 * PW, P], [512 * PW, full],
                                        [1, 4 * PW]]))
                if rem_p:
                    nc.sync.dma_start(
                        out=bass.AP(tensor=cfl.tensor,
                                    offset=cfl.offset + full * RW + 4 * YW,
                                    ap=[[pstride[0], rem_p], [1, 4 * PW]]),
                        in_=bass.AP(tensor=psrc.tensor,
                                    offset=pbase + full * 512 * PW,
                                    ap=[[4 * PW, rem_p], [1, 4 * PW]]))
                if odd:
                    nc.sync.dma_start(
                        out=CF[rem_p:rem_p + 1,
                               full * RW + 4 * YW:full * RW + 4 * YW +
                               odd * PW],
                        in_=bass.AP(
                            tensor=psrc.tensor,
                            offset=pbase + (full * 512 + 4 * rem_p) * PW,
                            ap=[[odd * PW, 1], [1, odd * PW]]))

                # fp32 front pred channels (conf, xy, wh) for dense decode
                FW = 20  # 4 cells x 5 ch
                PF = decp.tile([P, Gp * FW + 20], F32, tag="pf",
                               name=f"pf{li}_{img}")
                pfl = PF[:]
                pfsrc = pf_d[img]
                pfbase = pfsrc.offset + coff * 5
                if remc:
                    nc.vector.memset(
                        bass.AP(tensor=pfl.tensor,
                                offset=pfl.offset + (Gp - 1) * FW,
                                ap=[pfl.ap[0], [1, FW]]), PADV)
                nc.vector.memset(
                    bass.AP(tensor=pfl.tensor, offset=pfl.offset + Gp * FW,
                            ap=[pfl.ap[0], [1, 20]]), PADV)
                if full:
                    nc.sync.dma_start(
                        out=bass.AP(tensor=pfl.tensor, offset=pfl.offset,
                                    ap=[[pfl.ap[0][0], P], [FW, full],
                                        [1, FW]]),
                        in_=bass.AP(tensor=pfsrc.tensor, offset=pfbase,
                                    ap=[[FW, P], [512 * 5, full], [1, FW]]))
                if rem_p:
                    nc.sync.dma_start(
                        out=bass.AP(tensor=pfl.tensor,
                                    offset=pfl.offset + full * FW,
                                    ap=[[pfl.ap[0][0], rem_p], [1, FW]]),
                        in_=bass.AP(tensor=pfsrc.tensor,
                                    offset=pfbase + full * 512 * 5,
                                    ap=[[FW, rem_p], [1, FW]]))
                if odd:
                    nc.sync.dma_start(
                        out=PF[rem_p:rem_p + 1,
                               full * FW:full * FW + odd * 5],
                        in_=bass.AP(
                            tensor=pfsrc.tensor,
                            offset=pfbase + (full * 512 + 4 * rem_p) * 5,
                            ap=[[odd * 5, 1], [1, odd * 5]]))

                def pfv(c0, c1, _a=pfl, _g=Gp):
                    return bass.AP(tensor=_a.tensor, offset=_a.offset + c0,
                                   ap=[_a.ap[0], [FW, _g], [5, 4],
                                       [1, c1 - c0]])

                # compact copies of the interleaved dense channels
                OBJC = decp.tile([P, S], F32, tag="objc")
                oc = OBJC[:]
                nc.vector.tensor_copy(
                    bass.AP(tensor=oc.tensor, offset=oc.offset,
                            ap=[oc.ap[0], [4, Gp], [1, 4]]),
                    yv(0, 1).squeeze(3))
                XCF = decp.tile([P, S], F32, tag="xcf")
                xc = XCF[:]
                nc.scalar.copy(
                    bass.AP(tensor=xc.tensor, offset=xc.offset,
                            ap=[xc.ap[0], [4, Gp], [1, 4]]),
                    pfv(0, 1).squeeze(3))

                def compact2(tile):   # [P, Gp, 4, 2] view over [P, S, 2]
                    a = tile[:]
                    return bass.AP(tensor=a.tensor, offset=a.offset,
                                   ap=[a.ap[0], [8, Gp], [2, 4], [1, 2]])

                # ---- decode dense ----
                EXY = decp.tile([P, S, 2], F32, tag="exy")
                nc.scalar.activation(compact2(EXY), pfv(1, 3), ACT.Exp,
                                     scale=-1.0)
                nc.vector.tensor_scalar_add(EXY[:], EXY[:], 1.0)
                SGX = decp.tile([P, S, 2], F32, tag="sgx")
                nc.vector.reciprocal(SGX[:], EXY[:])
                CXY = decp.tile([P, S, 2], F32, tag="cxy")
                nc.vector.scalar_tensor_tensor(
                    CXY[:], SGX[:], 1.0 / W, GAD[:, goff:goff + S, 0:2],
                    ALU.mult, ALU.add)
                EWH = decp.tile([P, S, 2], F32, tag="ewh")
                nc.scalar.activation(compact2(EWH), pfv(3, 5), ACT.Exp)
                HWT = decp.tile([P, S, 2], F32, tag="hwt")
                nc.vector.tensor_mul(HWT[:], EWH[:],
                                     GAD[:, goff:goff + S, 2:4])
                PMX = decp.tile([P, S, 2], F16, tag="pmx")
                nc.vector.tensor_add(PMX[:], CXY[:], HWT[:])
                PMN = decp.tile([P, S, 2], F16, tag="pmn")
                nc.vector.tensor_sub(PMN[:], CXY[:], HWT[:])
                A13 = decp.tile([P, S], F16, tag="a13")
                nc.vector.scalar_tensor_tensor(
                    A13[:], HWT[:, :, 0], 4.0 / 3.0, HWT[:, :, 1],
                    ALU.mult, ALU.mult)

                # ---- rank & one-hot selection ----
                RCUM = decp.tile([P, S], F32, tag="rcum")
                nc.vector.tensor_tensor_scan(RCUM[:], OBJC[:], ZER[:, 0:S],
                                             0.0, ALU.add, ALU.add)
                OFFP = pso.tile([P, 1], F32, tag="offp")
                nc.tensor.matmul(OFFP[:], UT[:], RCUM[:, S - 1:S],
                                 start=True, stop=True)
                RANK = decp.tile([P, S], F32, tag="rank")
                nc.vector.tensor_scalar_add(RANK[:], RCUM[:], OFFP[:])
                RPM = decp.tile([P, S], F32, tag="rpm")
                nc.vector.tensor_mul(RPM[:], RANK[:], OBJC[:])
                STT = stp.tile([P, S, M], F16, tag="st")
                nc.vector.tensor_tensor(STT[:], bmid(IOTA[:, 0:M], S),
                                        RPM[:].broadcast_to([P, S, M]),
                                        ALU.is_equal)

                # ---- gather true rows (PE, fp16, 256-wide 2-block rhs) ----
                PGA = psg.tile([MM, 256], F32, tag="pga")
                for s in range(S):
                    g, j = s // 4, s % 4
                    yoff = g * RW + j * YW
                    delta = 4 * YW + j * PW - j * YW  # 356 - 4*j
                    rhs = bass.AP(tensor=cfl.tensor,
                                  offset=cfl.offset + yoff,
                                  ap=[[pstride[0], P], [delta, 2], [1, 128]])
                    nc.tensor.matmul(PGA[0:M, :], STT[:, s, :],
                                     rhs, start=(s == 0), stop=(s == S - 1))
                SPT = gatp.tile([MM, SW], F32, tag="spt")
                nc.scalar.copy(SPT[0:M, 0:YW], PGA[0:M, 0:YW])
                nc.scalar.copy(SPT[0:M, YW:SW], PGA[0:M, 128:128 + PW])
                nc.sync.dma_start(out=SPA[li][32 * img:32 * img + M, :],
                                  in_=SPT[0:M, :])
                # box rows (obj,x,y,w,h) -> dram -> [P,5,M] broadcast
                SCR = drp.tile([5, MM], F32, tag="scr")
                s1 = SPT[0:M, 0:5]
                s1t = bass.AP(tensor=s1.tensor, offset=s1.offset,
                              ap=[s1.ap[0], [1, 5], [1, 1]])
                d1 = bass.AP(tensor=SCR[:].tensor, offset=SCR[:].offset,
                             ap=[[1, M], [MM, 5], [1, 1]])
                nc.sync.dma_start(out=d1, in_=s1t)
                RAWB = gatp.tile([P, 5, MM], F32, tag="rawb")
                s2 = bass.AP(tensor=SCR[:].tensor, offset=SCR[:].offset,
                             ap=[[0, P], [1, 5 * MM]])
                d2 = bass.AP(tensor=RAWB[:].tensor, offset=RAWB[:].offset,
                             ap=[RAWB[:].ap[0], [1, 5 * MM]])
                nc.sync.dma_start(out=d2, in_=s2)

                BT = gatp.tile([P, 5, MM], F16, tag="bt")
                X_, Y_, W_, H_ = (RAWB[:, 1, 0:M], RAWB[:, 2, 0:M],
                                  RAWB[:, 3, 0:M], RAWB[:, 4, 0:M])
                nc.vector.scalar_tensor_tensor(BT[:, 0, 0:M], W_, -0.5, X_,
                                               ALU.mult, ALU.add)
                nc.vector.scalar_tensor_tensor(BT[:, 1, 0:M], W_, 0.5, X_,
                                               ALU.mult, ALU.add)
                nc.vector.scalar_tensor_tensor(BT[:, 2, 0:M], H_, -0.5, Y_,
                                               ALU.mult, ALU.add)
                nc.vector.scalar_tensor_tensor(BT[:, 3, 0:M], H_, 0.5, Y_,
                                               ALU.mult, ALU.add)
                nc.vector.scalar_tensor_tensor(BT[:, 4, 0:M], W_, 1.0 / 3.0,
                                               H_, ALU.mult, ALU.mult)

                # ---- IoU ignore: smax = max_m(rx*ry - a1/3 - a2/3) ----
                shp = [P, S, M]
                IX = ioup.tile(shp, F16, tag="ix")
                nc.vector.tensor_tensor(IX[:], PMX[:, :, 0].broadcast_to(shp),
                                        bmid(BT[:, 1, 0:M], S), ALU.min)
                JX = ioup.tile(shp, F16, tag="jx")
                nc.vector.tensor_tensor(JX[:], PMN[:, :, 0].broadcast_to(shp),
                                        bmid(BT[:, 0, 0:M], S), ALU.max)
                nc.vector.tensor_sub(IX[:], IX[:], JX[:])
                nc.vector.tensor_scalar_max(IX[:], IX[:], 0.0)
                IY = ioup.tile(shp, F16, tag="iy")
                nc.vector.tensor_tensor(IY[:], PMX[:, :, 1].broadcast_to(shp),
                                        bmid(BT[:, 3, 0:M], S), ALU.min)
                JY = ioup.tile(shp, F16, tag="jy")
                nc.vector.tensor_tensor(JY[:], PMN[:, :, 1].broadcast_to(shp),
                                        bmid(BT[:, 2, 0:M], S), ALU.max)
                nc.vector.tensor_sub(IY[:], IY[:], JY[:])
                nc.vector.tensor_scalar_max(IY[:], IY[:], 0.0)
                nc.vector.tensor_mul(JX[:], IX[:], IY[:])
                nc.vector.tensor_tensor(JX[:], JX[:],
                                        A13[:].broadcast_to(shp),
                                        ALU.subtract)
                nc.vector.tensor_tensor(JX[:], JX[:], bmid(BT[:, 4, 0:M], S),
                                        ALU.subtract)
                SMX = decp.tile([P, S], F32, tag="smx")
                nc.vector.tensor_reduce(SMX[:], JX[:], axis=AX.X, op=ALU.max)

                # ---- dense conf loss (on c = sigmoid(x)) ----
                WT = decp.tile([P, S], F32, tag="wt")
                nc.vector.scalar_tensor_tensor(WT[:], SMX[:], 0.0, OBJC[:],
                                               ALU.is_lt, ALU.max)
                nc.vector.tensor_mul(WT[:], WT[:], GAD[:, goff:goff + S, 4])
                ECF = decp.tile([P, S], F32, tag="ecf")
                nc.scalar.activation(ECF[:], XCF[:], ACT.Exp, scale=-1.0)
                nc.vector.tensor_scalar_add(ECF[:], ECF[:], 1.0)
                CCF = decp.tile([P, S], F32, tag="ccf")
                nc.vector.reciprocal(CCF[:], ECF[:])
                E3T = decp.tile([P, S], F32, tag="e3t")
                nc.scalar.activation(E3T[:], CCF[:], ACT.Exp, scale=-1.0)
                L1T = decp.tile([P, S], F32, tag="l1t")
                nc.scalar.activation(L1T[:], E3T[:], ACT.Ln, bias=1.0)
                SCRP = decp.tile([P, S], F32, tag="scrp")
                base = img * 9 + li * 3
                nc.vector.scalar_tensor_tensor(
                    SCRP[:], CCF[:], 1.0, WT[:], ALU.mult, ALU.mult,
                    accum_out=ACCD[:, base:base + 1])
                nc.vector.scalar_tensor_tensor(
                    SCRP[:], L1T[:], 1.0, WT[:], ALU.mult, ALU.mult,
                    accum_out=ACCD[:, base + 1:base + 2])
                nc.vector.scalar_tensor_tensor(
                    SCRP[:], CCF[:], 1.0, OBJC[:], ALU.mult, ALU.mult,
                    accum_out=ACCD[:, base + 2:base + 3])

        # ---- sparse losses per layer (4 images batched on partitions) ----
        for li, lay in enumerate(LAYERS):
            W = lay["W"]
            Sp = SPA[li]
            obj = Sp[:, 0:1]
            WH1 = spap.tile([P, 1], F32, tag="wh1")
            nc.vector.tensor_mul(WH1[:], Sp[:, 3:4], Sp[:, 4:5])
            SC = spap.tile([P, 1], F32, tag="sc")
            nc.vector.tensor_scalar(SC[:], WH1[:], -1.0, 2.0, ALU.mult,
                                    ALU.add)
            OSC = spap.tile([P, 1], F32, tag="osc")
            nc.vector.tensor_mul(OSC[:], SC[:], obj)
            IV = spap.tile([P, 1], F32, tag="iv")
            nc.vector.tensor_scalar(IV[:], obj, -1.0, 1.0, ALU.mult, ALU.add)
            # xy
            EX = spap.tile([P, 2], F32, tag="ex")
            nc.scalar.activation(EX[:], Sp[:, 90:92], ACT.Exp, scale=-1.0)
            nc.vector.tensor_scalar_add(EX[:], EX[:], 1.0)
            SG = spap.tile([P, 2], F32, tag="sg")
            nc.vector.reciprocal(SG[:], EX[:])
            CX = spap.tile([P, 2], F32, tag="cx")
            nc.vector.tensor_add(CX[:], SG[:], Sp[:, 85:87])
            nc.vector.tensor_scalar_mul(CX[:], CX[:], 1.0 / W)
            TX = spap.tile([P, 2], F32, tag="tx")
            nc.vector.scalar_tensor_tensor(TX[:], Sp[:, 1:3], W, Sp[:, 85:87],
                                           ALU.mult, ALU.subtract)
            EB = spap.tile([P, 2], F32, tag="eb")
            nc.scalar.activation(EB[:], CX[:], ACT.Exp, scale=-1.0)
            LB = spap.tile([P, 2], F32, tag="lb")
            nc.scalar.activation(LB[:], EB[:], ACT.Ln, bias=1.0)
            OMT = spap.tile([P, 2], F32, tag="omt")
            nc.vector.tensor_scalar(OMT[:], TX[:], -1.0, 1.0, ALU.mult,
                                    ALU.add)
            VV = spap.tile([P, 2], F32, tag="vv")
            nc.vector.tensor_mul(VV[:], OMT[:], CX[:])
            nc.vector.tensor_add(VV[:], VV[:], LB[:])
            SCR2 = spap.tile([P, 2], F32, tag="scr2")
            nc.vector.tensor_scalar(SCR2[:], VV[:], OSC[:], 0.0, ALU.mult,
                                    ALU.add,
                                    accum_out=SACC[:, 3 * li:3 * li + 1])
            # wh
            T1 = spap.tile([P, 2], F32, tag="t1")
            nc.vector.tensor_scalar(T1[:], Sp[:, 3:5], IMG_W, IV[:], ALU.mult,
                                    ALU.add)
            nc.scalar.activation(T1[:], T1[:], ACT.Ln)
            T2 = spap.tile([P, 2], F32, tag="t2")
            nc.vector.tensor_scalar_add(T2[:], Sp[:, 87:89], IV[:])
            nc.scalar.activation(T2[:], T2[:], ACT.Ln)
            nc.vector.tensor_sub(T1[:], T1[:], T2[:])   # true_wh
            EW2 = spap.tile([P, 2], F32, tag="ew2")
            nc.scalar.activation(EW2[:], Sp[:, 92:94], ACT.Exp)
            AN = spap.tile([P, 2], F32, tag="an")
            nc.vector.tensor_scalar_mul(AN[:], Sp[:, 87:89], 1.0 / W)
            nc.vector.tensor_mul(EW2[:], EW2[:], AN[:])  # pred wh
            nc.vector.tensor_sub(T1[:], T1[:], EW2[:])
            DW2 = spap.tile([P, 2], F32, tag="dw2")
            nc.scalar.activation(DW2[:], T1[:], ACT.Square)
            OSC5 = spap.tile([P, 1], F32, tag="osc5")
            nc.vector.tensor_scalar_mul(OSC5[:], OSC[:], 0.5)
            nc.vector.tensor_scalar(SCR2[:], DW2[:], OSC5[:], 0.0, ALU.mult,
                                    ALU.add,
                                    accum_out=SACC[:, 3 * li + 1:3 * li + 2])
            # cls
            EC = spap.tile([P, 80], F32, tag="ec")
            nc.scalar.activation(EC[:], Sp[:, 94:174], ACT.Exp, scale=-1.0)
            nc.vector.tensor_scalar_add(EC[:], EC[:], 1.0)
            SGC = spap.tile([P, 80], F32, tag="sgc")
            nc.vector.reciprocal(SGC[:], EC[:])
            EB2 = spap.tile([P, 80], F32, tag="eb2")
            nc.scalar.activation(EB2[:], SGC[:], ACT.Exp, scale=-1.0)
            LB2 = spap.tile([P, 80], F32, tag="lb2")
            nc.scalar.activation(LB2[:], EB2[:], ACT.Ln, bias=1.0)
            OM2 = spap.tile([P, 80], F32, tag="om2")
            nc.vector.tensor_scalar(OM2[:], Sp[:, 5:85], -1.0, 1.0, ALU.mult,
                                    ALU.add)
            nc.vector.tensor_mul(OM2[:], OM2[:], SGC[:])
            nc.vector.tensor_add(OM2[:], OM2[:], LB2[:])
            SCR3 = spap.tile([P, 80], F32, tag="scr3")
            nc.vector.tensor_scalar(SCR3[:], OM2[:], obj, 0.0, ALU.mult,
                                    ALU.add,
                                    accum_out=SACC[:, 3 * li + 2:3 * li + 3])

        # ---- final combine ----
        AC3 = ACCD[:].rearrange("p (x t) -> p x t", t=3)
        TMP = accp.tile([P, B_CORE * 3], F32)
        nc.vector.tensor_add(TMP[:], AC3[:, :, 0], AC3[:, :, 1])
        nc.vector.tensor_tensor(TMP[:], TMP[:], AC3[:, :, 2], ALU.subtract)
        FIN = accp.tile([P, B_CORE], F32)
        nc.vector.tensor_reduce(
            FIN[:], TMP[:].rearrange("p (i l) -> p i l", l=3),
            axis=AX.X, op=ALU.add)
        FSP = accp.tile([P, 1], F32)
        nc.vector.tensor_reduce(FSP[:], SACC[:], axis=AX.X, op=ALU.add)
        PL = pso.tile([B_CORE, 1], F32, tag="pl")
        nc.tensor.matmul(PL[:], FIN[:], ON128[:], start=True, stop=False)
        nc.tensor.matmul(PL[:], SELC[:], FSP[:], start=False, stop=True)
        OUT = accp.tile([B_CORE, 1], F32)
        nc.scalar.copy(OUT[:], PL[:])
        nc.sync.dma_start(out=loss_d[:], in_=OUT[:])

    nc.finalize()
    return nc


def _prep_core_inputs(y_true, pred_13, pred_26, pred_52):
    consts, percell = _make_consts()
    yt85 = np.asarray(y_true).reshape(32, 10647, 85)
    yt = np.empty((32, 10647, YW), np.float16)
    yt[:, :, 0:85] = yt85
    yt[:, :, 85:89] = percell[None]
    ps32 = [np.asarray(p).reshape(32, -1, 85)
            for p in (pred_13, pred_26, pred_52)]
    ps = [np.ascontiguousarray(p.astype(np.float16)) for p in ps32]
    pf = np.ascontiguousarray(
        np.concatenate([p[:, :, 0:5] for p in ps32], axis=1))
    in_maps = []
    for c in range(N_CORES):
        sl = slice(c * B_CORE, (c + 1) * B_CORE)
        m = {"yt": yt[sl], "p0": ps[0][sl], "p1": ps[1][sl],
             "p2": ps[2][sl], "pf": pf[sl]}
        m.update(consts)
        in_maps.append(m)
    return in_maps


def kernel(y_true, pred_13, pred_26, pred_52):
    from concourse.bass_utils import run_bass_kernel_spmd

    Ms = [8, 16, 32]
    obj = np.asarray(y_true)[..., 0].reshape(32, 10647)
    cnt = [obj[:, LAYERS[i]["coff"]:LAYERS[i]["coff"] + LAYERS[i]["N"]]
           .sum(1).max() for i in range(3)]
    if any(cnt[i] > Ms[i] for i in range(3)):
        Ms = [64, 64, 64]
    key = tuple(Ms)
    if key not in _NC_CACHE:
        _NC_CACHE[key] = build_nc(Ms)
    nc = _NC_CACHE[key]

    in_maps = _prep_core_inputs(y_true, pred_13, pred_26, pred_52)
    res = run_bass_kernel_spmd(nc, in_maps, core_ids=list(range(N_CORES)))
    out = np.concatenate([r["loss"].reshape(B_CORE) for r in res.results])
    return out.astype(np.float32)


# revision 29
# speedup vs baseline: 1.1430x; 1.0741x over previous
"""YOLO loss (nms_detection) Trainium2 Bass kernel.

Data parallel over 8 NeuronCores (4 images per core). Per (image, layer):
  - y_true is host-augmented with per-cell (gx, gy, aw, ah) -> 89 channels,
    so one TensorEngine gather fetches labels + grid + anchors together.
  - inputs are host-cast to fp16 and cells quad-packed per partition row
    so every DMA descriptor moves >= 680B contiguous at half the bytes.
  - decode pred boxes (sigmoid via exp+reciprocal; one ACT table set).
  - obj compaction: row cumsum (tensor_tensor_scan) + triangular-matmul
    partition offsets -> rank; one-hot S = (iota == rank*obj).
  - gather true-box rows via fp16 matmuls (256-wide two-block rhs);
    dense decode reads a small fp32 copy of the conf/xy/wh channels.
  - broadcast box quantities via DRAM-roundtrip DMA.
  - IoU ignore mask in fp16: big [128, S, M] broadcast-AP DVE ops testing
    3*inter >= a1+a2 (equiv. IoU >= 0.5, no division).
  - dense conf BCE on c = sigmoid(x) with weight max(obj, ignore)*valid;
    obj-masked xy/wh/cls losses on the gathered [M, 174] rows only.
"""

from contextlib import ExitStack

import numpy as np

ANCHORS = np.array([[116., 90.], [156., 198.], [373., 326.],
                    [30., 61.], [62., 45.], [59., 119.],
                    [10., 13.], [16., 30.], [33., 23.]], dtype=np.float32)
IMG_W = 416.0
P = 128
B_CORE = 4
N_CORES = 8
YW = 89           # augmented y_true row: 85 + (gx, gy, aw, ah)
PW = 85
RW = 4 * YW + 4 * PW   # 696: [ytA..ytD | predA..predD] (fp16, quad cells)
TAILPAD = 48      # gather rhs reads up to row_base + 696 + 43 -> pad 48
SW = 174          # gathered sparse row: yt_aug 89 + pred 85
PADV = -60.0

# per-layer: N cells, slots S (=ceil(N/128) padded even), grid W, offsets
LAYERS = [
    dict(N=507,  S=4,  W=13.0, coff=0,    goff=0),
    dict(N=2028, S=16, W=26.0, coff=507,  goff=4),
    dict(N=8112, S=64, W=52.0, coff=2535, goff=20),
]
STOT = 84

_NC_CACHE = {}


def _make_consts():
    # dense grid/anchor const: (gxw, gyw, awhalf, ahhalf, valid)
    gad = np.zeros((P, STOT, 5), np.float32)
    # per-cell ga columns appended to y_true, in flat cell order
    percell = np.zeros((10647, 4), np.float32)
    for li, lay in enumerate(LAYERS):
        W = int(lay["W"])
        N, S, goff, coff = lay["N"], lay["S"], lay["goff"], lay["coff"]
        c = np.arange(N)
        percell[coff:coff + N, 0] = (c % (W * 3)) // 3
        percell[coff:coff + N, 1] = c // (W * 3)
        percell[coff:coff + N, 2] = ANCHORS[3 * li + (c % 3), 0]
        percell[coff:coff + N, 3] = ANCHORS[3 * li + (c % 3), 1]
        p = np.arange(P)[:, None]
        s = np.arange(S)[None, :]
        cell = (s // 4) * 512 + 4 * p + (s % 4)
        valid = cell < N
        cc = np.minimum(cell, N - 1)
        aw = ANCHORS[3 * li + (cc % 3), 0]
        ah = ANCHORS[3 * li + (cc % 3), 1]
        gx = ((cc % (W * 3)) // 3).astype(np.float32)
        gy = (cc // (W * 3)).astype(np.float32)
        gad[:, goff:goff + S, 0] = np.where(valid, gx / W, 0)
        gad[:, goff:goff + S, 1] = np.where(valid, gy / W, 0)
        gad[:, goff:goff + S, 2] = np.where(valid, aw / (2.0 * W), 0)
        gad[:, goff:goff + S, 3] = np.where(valid, ah / (2.0 * W), 0)
        gad[:, goff:goff + S, 4] = valid.astype(np.float32)
    ut = np.triu(np.ones((P, P), np.float32), 1)  # ut[q,p]=1 iff q<p
    ones128 = np.ones((P, 1), np.float32)
    sel = np.zeros((P, B_CORE), np.float32)
    for i in range(B_CORE):
        sel[32 * i:32 * (i + 1), i] = 1.0
    return {"gad": gad, "ut": ut, "ones128": ones128, "sel": sel}, percell


def build_nc(Ms):
    import concourse.bass as bass
    import concourse.bacc as bacc
    import concourse.mybir as mybir
    from concourse.tile import TileContext

    F32 = mybir.dt.float32
    F16 = mybir.dt.float16
    F32R = mybir.dt.float32r
    ALU = mybir.AluOpType
    ACT = mybir.ActivationFunctionType
    AX = mybir.AxisListType
    MM = max(Ms)

    nc = bacc.Bacc()
    yt_d = nc.dram_tensor("yt", [B_CORE, 10647, YW], F16,
                          kind="ExternalInput")
    pr_d = [nc.dram_tensor(f"p{i}", [B_CORE, LAYERS[i]["N"], PW], F16,
                           kind="ExternalInput") for i in range(3)]
    pf_d = nc.dram_tensor("pf", [B_CORE, 10647, 5], F32,
                          kind="ExternalInput")
    ga_d = nc.dram_tensor("gad", [P, STOT, 5], F32, kind="ExternalInput")
    ut_d = nc.dram_tensor("ut", [P, P], F32, kind="ExternalInput")
    on_d = nc.dram_tensor("ones128", [P, 1], F32, kind="ExternalInput")
    se_d = nc.dram_tensor("sel", [P, B_CORE], F32, kind="ExternalInput")
    loss_d = nc.dram_tensor("loss", [B_CORE, 1], F32, kind="ExternalOutput")

    def bmid(ap2, n):
        # [P, X] -> [P, n, X] (step-0 middle dim)
        return bass.AP(tensor=ap2.tensor, offset=ap2.offset,
                       ap=[ap2.ap[0], [0, n]] + ap2.ap[1:])

    big = MM > 32   # fallback config must fit SBUF with M=64
    with TileContext(nc) as tc, ExitStack() as ctx:
        cpool = ctx.enter_context(tc.tile_pool(name="consts", bufs=1))
        combp = {li: ctx.enter_context(
            tc.tile_pool(name=f"comb{li}",
                         bufs=1 if (big and li == 2) else 3))
                 for li in range(3)}
        decp = ctx.enter_context(tc.tile_pool(name="dec", bufs=2 if big else 4))
        ioup = ctx.enter_context(tc.tile_pool(name="iou", bufs=1 if big else 2))
        stp = ctx.enter_context(tc.tile_pool(name="st", bufs=2 if big else 4))
        gatp = ctx.enter_context(
            tc.tile_pool(name="gat", bufs=2 if big else 3))
        spap = ctx.enter_context(tc.tile_pool(name="spa", bufs=1))
        accp = ctx.enter_context(tc.tile_pool(name="acc", bufs=1))
        drp = ctx.enter_context(
            tc.tile_pool(name="scr", bufs=3, space=bass.MemorySpace.DRAM))
        psg = ctx.enter_context(
            tc.tile_pool(name="psg", bufs=3, space=bass.MemorySpace.PSUM))
        pso = ctx.enter_context(
            tc.tile_pool(name="pso", bufs=2, space=bass.MemorySpace.PSUM))

        GAD = cpool.tile([P, STOT, 5], F32)
        nc.sync.dma_start(out=GAD, in_=ga_d[:])
        UT = cpool.tile([P, P], F32)
        nc.sync.dma_start(out=UT, in_=ut_d[:])
        ON128 = cpool.tile([P, 1], F32)
        nc.sync.dma_start(out=ON128, in_=on_d[:])
        SELC = cpool.tile([P, B_CORE], F32)
        nc.sync.dma_start(out=SELC, in_=se_d[:])
        IOTA = cpool.tile([P, MM], F32)
        nc.gpsimd.iota(IOTA[:], [[1, MM]], base=1, channel_multiplier=0,
                       allow_small_or_imprecise_dtypes=True)
        ZER = cpool.tile([P, 64], F32)
        nc.gpsimd.memset(ZER[:], 0.0)

        ACCD = accp.tile([P, B_CORE * 9], F32)   # (img, layer, term) dense
        SACC = accp.tile([P, 9], F32)            # (layer, term) sparse
        nc.gpsimd.memset(SACC[:], 0.0)
        SPA = {li: spap.tile([P, SW], F32, tag=f"spa{li}", name=f"spa{li}")
               for li in range(3)}
        for li in range(3):
            nc.gpsimd.memset(SPA[li][:], 0.0)

        for img in range(B_CORE):
            for li in (2, 1, 0):
                lay = LAYERS[li]
                N, S, W, coff, goff = (lay["N"], lay["S"], lay["W"],
                                       lay["coff"], lay["goff"])
                M = Ms[li]
                Gp = S // 4                # quad rows
                full = N // 512            # full quad rows
                remc = N - full * 512
                rem_p = remc // 4
                odd = remc % 4             # 0..3 extra cells on one partition
                CF = combp[li].tile([P, Gp * RW + TAILPAD], F16,
                                    tag=f"comb{li}", name=f"comb{li}_{img}")
                cfl = CF[:]
                pstride = cfl.ap[0]

                def yv(c0, c1, _a=cfl, _g=Gp):
                    # yt view [P, Gp, 4, c1-c0]
                    return bass.AP(tensor=_a.tensor, offset=_a.offset + c0,
                                   ap=[_a.ap[0], [RW, _g], [YW, 4],
                                       [1, c1 - c0]])

                def pv(c0, c1, _a=cfl, _g=Gp):
                    return bass.AP(tensor=_a.tensor,
                                   offset=_a.offset + 4 * YW + c0,
                                   ap=[_a.ap[0], [RW, _g], [PW, 4],
                                       [1, c1 - c0]])

                def cview(off, n, _a=cfl):
                    return bass.AP(tensor=_a.tensor, offset=_a.offset + off,
                                   ap=[_a.ap[0], [1, n]])

                # pad init: tail cols + last quad row (dma overwrites live)
                nc.vector.memset(cview(Gp * RW, TAILPAD), 0.0)
                if remc:
                    nc.vector.memset(cview((Gp - 1) * RW, 4 * YW), 0.0)
                    nc.vector.memset(cview((Gp - 1) * RW + 4 * YW, 4 * PW),
                                     PADV)
                # ---- loads (contiguous >= 680B elements) ----
                ysrc = yt_d[img]
                ybase = ysrc.offset + coff * YW
                if full:
                    nc.sync.dma_start(
                        out=bass.AP(tensor=cfl.tensor, offset=cfl.offset,
                                    ap=[[pstride[0], P], [RW, full],
                                        [1, 4 * YW]]),
                        in_=bass.AP(tensor=ysrc.tensor, offset=ybase,
                                    ap=[[4 * YW, P], [512 * YW, full],
                                        [1, 4 * YW]]))
                if rem_p:
                    nc.sync.dma_start(
                        out=bass.AP(tensor=cfl.tensor,
                                    offset=cfl.offset + full * RW,
                                    ap=[[pstride[0], rem_p], [1, 4 * YW]]),
                        in_=bass.AP(tensor=ysrc.tensor,
                                    offset=ybase + full * 512 * YW,
                                    ap=[[4 * YW, rem_p], [1, 4 * YW]]))
                if odd:
                    nc.sync.dma_start(
                        out=CF[rem_p:rem_p + 1,
                               full * RW:full * RW + odd * YW],
                        in_=bass.AP(
                            tensor=ysrc.tensor,
                            offset=ybase + (full * 512 + 4 * rem_p) * YW,
                            ap=[[odd * YW, 1], [1, odd * YW]]))
                psrc = pr_d[li][img]
                pbase = psrc.offset
                if full:
                    nc.sync.dma_start(
                        out=bass.AP(tensor=cfl.tensor,
                                    offset=cfl.offset + 4 * YW,
                                    ap=[[pstride[0], P], [RW, full],
                                        [1, 4 * PW]]),
                        in_=bass.AP(tensor=psrc.tensor, offset=pbase,
                                    ap=[[4 * PW, P], [512 * PW, full],
                                        [1, 4 * PW]]))
                if rem_p:
                    nc.sync.dma_start(
                        out=bass.AP(tensor=cfl.tensor,
                                    offset=cfl.offset + full * RW + 4 * YW,
                                    ap=[[pstride[0], rem_p], [1, 4 * PW]]),
                        in_=bass.AP(tensor=psrc.tensor,
                                    offset=pbase + full * 512 * PW,
                                    ap=[[4 * PW, rem_p], [1, 4 * PW]]))
                if odd:
                    nc.sync.dma_start(
                        out=CF[rem_p:rem_p + 1,
                               full * RW + 4 * YW:full * RW + 4 * YW +
                               odd * PW],
                        in_=bass.AP(
                            tensor=psrc.tensor,
                            offset=pbase + (full * 512 + 4 * rem_p) * PW,
                            ap=[[odd * PW, 1], [1, odd * PW]]))

                # fp32 front pred channels (conf, xy, wh) for dense decode
                FW = 20  # 4 cells x 5 ch
                PF = decp.tile([P, Gp * FW + 20], F32, tag="pf",
                               name=f"pf{li}_{img}")
                pfl = PF[:]
                pfsrc = pf_d[img]
                pfbase = pfsrc.offset + coff * 5
                if remc:
                    nc.vector.memset(
                        bass.AP(tensor=pfl.tensor,
                                offset=pfl.offset + (Gp - 1) * FW,
                                ap=[pfl.ap[0], [1, FW]]), PADV)
                nc.vector.memset(
                    bass.AP(tensor=pfl.tensor, offset=pfl.offset + Gp * FW,
                            ap=[pfl.ap[0], [1, 20]]), PADV)
                if full:
                    nc.sync.dma_start(
                        out=bass.AP(tensor=pfl.tensor, offset=pfl.offset,
                                    ap=[[pfl.ap[0][0], P], [FW, full],
                                        [1, FW]]),
                        in_=bass.AP(tensor=pfsrc.tensor, offset=pfbase,
                                    ap=[[FW, P], [512 * 5, full], [1, FW]]))
                if rem_p:
                    nc.sync.dma_start(
                        out=bass.AP(tensor=pfl.tensor,
                                    offset=pfl.offset + full * FW,
                                    ap=[[pfl.ap[0][0], rem_p], [1, FW]]),
                        in_=bass.AP(tensor=pfsrc.tensor,
                                    offset=pfbase + full * 512 * 5,
                                    ap=[[FW, rem_p], [1, FW]]))
                if odd:
                    nc.sync.dma_start(
                        out=PF[rem_p:rem_p + 1,
                               full * FW:full * FW + odd * 5],
                        in_=bass.AP(
                            tensor=pfsrc.tensor,
                            offset=pfbase + (full * 512 + 4 * rem_p) * 5,
                            ap=[[odd * 5, 1], [1, odd * 5]]))

                def pfv(c0, c1, _a=pfl, _g=Gp):
                    return bass.AP(tensor=_a.tensor, offset=_a.offset + c0,
                                   ap=[_a.ap[0], [FW, _g], [5, 4],
                                       [1, c1 - c0]])

                # compact copies of the interleaved dense channels
                OBJC = decp.tile([P, S], F32, tag="objc")
                oc = OBJC[:]
                nc.vector.tensor_copy(
                    bass.AP(tensor=oc.tensor, offset=oc.offset,
                            ap=[oc.ap[0], [4, Gp], [1, 4]]),
                    yv(0, 1).squeeze(3))
                XCF = decp.tile([P, S], F32, tag="xcf")
                xc = XCF[:]
                nc.scalar.copy(
                    bass.AP(tensor=xc.tensor, offset=xc.offset,
                            ap=[xc.ap[0], [4, Gp], [1, 4]]),
                    pfv(0, 1).squeeze(3))

                def compact2(tile):   # [P, Gp, 4, 2] view over [P, S, 2]
                    a = tile[:]
                    return bass.AP(tensor=a.tensor, offset=a.offset,
                                   ap=[a.ap[0], [8, Gp], [2, 4], [1, 2]])

                # ---- decode dense ----
                EXY = decp.tile([P, S, 2], F32, tag="exy")
                nc.scalar.activation(compact2(EXY), pfv(1, 3), ACT.Exp,
                                     scale=-1.0)
                nc.vector.tensor_scalar_add(EXY[:], EXY[:], 1.0)
                SGX = decp.tile([P, S, 2], F32, tag="sgx")
                nc.vector.reciprocal(SGX[:], EXY[:])
                CXY = decp.tile([P, S, 2], F32, tag="cxy")
                nc.vector.scalar_tensor_tensor(
                    CXY[:], SGX[:], 1.0 / W, GAD[:, goff:goff + S, 0:2],
                    ALU.mult, ALU.add)
                EWH = decp.tile([P, S, 2], F32, tag="ewh")
                nc.scalar.activation(compact2(EWH), pfv(3, 5), ACT.Exp)
                HWT = decp.tile([P, S, 2], F32, tag="hwt")
                nc.vector.tensor_mul(HWT[:], EWH[:],
                                     GAD[:, goff:goff + S, 2:4])
                PMX = decp.tile([P, S, 2], F16, tag="pmx")
                nc.vector.tensor_add(PMX[:], CXY[:], HWT[:])
                PMN = decp.tile([P, S, 2], F16, tag="pmn")
                nc.vector.tensor_sub(PMN[:], CXY[:], HWT[:])
                A13 = decp.tile([P, S], F16, tag="a13")
                nc.vector.scalar_tensor_tensor(
                    A13[:], HWT[:, :, 0], 4.0 / 3.0, HWT[:, :, 1],
                    ALU.mult, ALU.mult)

                # ---- rank & one-hot selection ----
                RCUM = decp.tile([P, S], F32, tag="rcum")
                nc.vector.tensor_tensor_scan(RCUM[:], OBJC[:], ZER[:, 0:S],
                                             0.0, ALU.add, ALU.add)
                OFFP = pso.tile([P, 1], F32, tag="offp")
                nc.tensor.matmul(OFFP[:], UT[:], RCUM[:, S - 1:S],
                                 start=True, stop=True)
                RANK = decp.tile([P, S], F32, tag="rank")
                nc.vector.tensor_scalar_add(RANK[:], RCUM[:], OFFP[:])
                RPM = decp.tile([P, S], F32, tag="rpm")
                nc.vector.tensor_mul(RPM[:], RANK[:], OBJC[:])
                STT = stp.tile([P, S, M], F16, tag="st")
                nc.vector.tensor_tensor(STT[:], bmid(IOTA[:, 0:M], S),
                                        RPM[:].broadcast_to([P, S, M]),
                                        ALU.is_equal)

                # ---- gather true rows (PE, fp16, 256-wide 2-block rhs) ----
                PGA = psg.tile([MM, 256], F32, tag="pga")
                for s in range(S):
                    g, j = s // 4, s % 4
                    yoff = g * RW + j * YW
                    delta = 4 * YW + j * PW - j * YW  # 356 - 4*j
                    rhs = bass.AP(tensor=cfl.tensor,
                                  offset=cfl.offset + yoff,
                                  ap=[[pstride[0], P], [delta, 2], [1, 128]])
                    nc.tensor.matmul(PGA[0:M, :], STT[:, s, :],
                                     rhs, start=(s == 0), stop=(s == S - 1))
                SPT = gatp.tile([MM, SW], F32, tag="spt")
                nc.scalar.copy(SPT[0:M, 0:YW], PGA[0:M, 0:YW])
                nc.scalar.copy(SPT[0:M, YW:SW], PGA[0:M, 128:128 + PW])
                nc.sync.dma_start(out=SPA[li][32 * img:32 * img + M, :],
                                  in_=SPT[0:M, :])
                # box rows (obj,x,y,w,h) -> dram -> [P,5,M] broadcast
                SCR = drp.tile([5, MM], F32, tag="scr")
                s1 = SPT[0:M, 0:5]
                s1t = bass.AP(tensor=s1.tensor, offset=s1.offset,
                              ap=[s1.ap[0], [1, 5], [1, 1]])
                d1 = bass.AP(tensor=SCR[:].tensor, offset=SCR[:].offset,
                             ap=[[1, M], [MM, 5], [1, 1]])
                nc.sync.dma_start(out=d1, in_=s1t)
                RAWB = gatp.tile([P, 5, MM], F32, tag="rawb")
                s2 = bass.AP(tensor=SCR[:].tensor, offset=SCR[:].offset,
                             ap=[[0, P], [1, 5 * MM]])
                d2 = bass.AP(tensor=RAWB[:].tensor, offset=RAWB[:].offset,
                             ap=[RAWB[:].ap[0], [1, 5 * MM]])
                nc.sync.dma_start(out=d2, in_=s2)

                BT = gatp.tile([P, 5, MM], F16, tag="bt")
                X_, Y_, W_, H_ = (RAWB[:, 1, 0:M], RAWB[:, 2, 0:M],
                                  RAWB[:, 3, 0:M], RAWB[:, 4, 0:M])
                nc.vector.scalar_tensor_tensor(BT[:, 0, 0:M], W_, -0.5, X_,
                                               ALU.mult, ALU.add)
                nc.vector.scalar_tensor_tensor(BT[:, 1, 0:M], W_, 0.5, X_,
                                               ALU.mult, ALU.add)
                nc.vector.scalar_tensor_tensor(BT[:, 2, 0:M], H_, -0.5, Y_,
                                               ALU.mult, ALU.add)
                nc.vector.scalar_tensor_tensor(BT[:, 3, 0:M], H_, 0.5, Y_,
                                               ALU.mult, ALU.add)
                nc.vector.scalar_tensor_tensor(BT[:, 4, 0:M], W_, 1.0 / 3.0,
                                               H_, ALU.mult, ALU.mult)

                # ---- IoU ignore: smax = max_m(rx*ry - a1/3 - a2/3) ----
                shp = [P, S, M]
                IX = ioup.tile(shp, F16, tag="ix")
                nc.vector.tensor_tensor(IX[:], PMX[:, :, 0].broadcast_to(shp),
                                        bmid(BT[:, 1, 0:M], S), ALU.min)
                JX = ioup.tile(shp, F16, tag="jx")
                nc.vector.tensor_tensor(JX[:], PMN[:, :, 0].broadcast_to(shp),
                                        bmid(BT[:, 0, 0:M], S), ALU.max)
                nc.vector.tensor_sub(IX[:], IX[:], JX[:])
                nc.scalar.activation(IX[:], IX[:], ACT.Relu)
                IY = ioup.tile(shp, F16, tag="iy")
                nc.vector.tensor_tensor(IY[:], PMX[:, :, 1].broadcast_to(shp),
                                        bmid(BT[:, 3, 0:M], S), ALU.min)
                JY = ioup.tile(shp, F16, tag="jy")
                nc.vector.tensor_tensor(JY[:], PMN[:, :, 1].broadcast_to(shp),
                                        bmid(BT[:, 2, 0:M], S), ALU.max)
                nc.vector.tensor_sub(IY[:], IY[:], JY[:])
                nc.scalar.activation(IY[:], IY[:], ACT.Relu)
                nc.vector.tensor_mul(JX[:], IX[:], IY[:])
                nc.vector.tensor_tensor(JX[:], JX[:],
                                        A13[:].broadcast_to(shp),
                                        ALU.subtract)
                nc.vector.tensor_tensor(JX[:], JX[:], bmid(BT[:, 4, 0:M], S),
                                        ALU.subtract)
                SMX = decp.tile([P, S], F32, tag="smx")
                nc.vector.tensor_reduce(SMX[:], JX[:], axis=AX.X, op=ALU.max)

                # ---- dense conf loss (on c = sigmoid(x)) ----
                WT = decp.tile([P, S], F32, tag="wt")
                nc.vector.scalar_tensor_tensor(WT[:], SMX[:], 0.0, OBJC[:],
                                               ALU.is_lt, ALU.max)
                nc.vector.tensor_mul(WT[:], WT[:], GAD[:, goff:goff + S, 4])
                ECF = decp.tile([P, S], F32, tag="ecf")
                nc.scalar.activation(ECF[:], XCF[:], ACT.Exp, scale=-1.0)
                nc.vector.tensor_scalar_add(ECF[:], ECF[:], 1.0)
                CCF = decp.tile([P, S], F32, tag="ccf")
                nc.vector.reciprocal(CCF[:], ECF[:])
                E3T = decp.tile([P, S], F32, tag="e3t")
                nc.scalar.activation(E3T[:], CCF[:], ACT.Exp, scale=-1.0)
                L1T = decp.tile([P, S], F32, tag="l1t")
                nc.scalar.activation(L1T[:], E3T[:], ACT.Ln, bias=1.0)
                SCRP = decp.tile([P, S], F32, tag="scrp")
                base = img * 9 + li * 3
                nc.vector.scalar_tensor_tensor(
                    SCRP[:], CCF[:], 1.0, WT[:], ALU.mult, ALU.mult,
                    accum_out=ACCD[:, base:base + 1])
                nc.vector.scalar_tensor_tensor(
                    SCRP[:], L1T[:], 1.0, WT[:], ALU.mult, ALU.mult,
                    accum_out=ACCD[:, base + 1:base + 2])
                nc.vector.scalar_tensor_tensor(
                    SCRP[:], CCF[:], 1.0, OBJC[:], ALU.mult, ALU.mult,
                    accum_out=ACCD[:, base + 2:base + 3])

        # ---- sparse losses per layer (4 images batched on partitions) ----
        for li, lay in enumerate(LAYERS):
            W = lay["W"]
            Sp = SPA[li]
            obj = Sp[:, 0:1]
            WH1 = spap.tile([P, 1], F32, tag="wh1")
            nc.vector.tensor_mul(WH1[:], Sp[:, 3:4], Sp[:, 4:5])
            SC = spap.tile([P, 1], F32, tag="sc")
            nc.vector.tensor_scalar(SC[:], WH1[:], -1.0, 2.0, ALU.mult,
                                    ALU.add)
            OSC = spap.tile([P, 1], F32, tag="osc")
            nc.vector.tensor_mul(OSC[:], SC[:], obj)
            IV = spap.tile([P, 1], F32, tag="iv")
            nc.vector.tensor_scalar(IV[:], obj, -1.0, 1.0, ALU.mult, ALU.add)
            # xy
            EX = spap.tile([P, 2], F32, tag="ex")
            nc.scalar.activation(EX[:], Sp[:, 90:92], ACT.Exp, scale=-1.0)
            nc.vector.tensor_scalar_add(EX[:], EX[:], 1.0)
            SG = spap.tile([P, 2], F32, tag="sg")
            nc.vector.reciprocal(SG[:], EX[:])
            CX = spap.tile([P, 2], F32, tag="cx")
            nc.vector.tensor_add(CX[:], SG[:], Sp[:, 85:87])
            nc.vector.tensor_scalar_mul(CX[:], CX[:], 1.0 / W)
            TX = spap.tile([P, 2], F32, tag="tx")
            nc.vector.scalar_tensor_tensor(TX[:], Sp[:, 1:3], W, Sp[:, 85:87],
                                           ALU.mult, ALU.subtract)
            EB = spap.tile([P, 2], F32, tag="eb")
            nc.scalar.activation(EB[:], CX[:], ACT.Exp, scale=-1.0)
            LB = spap.tile([P, 2], F32, tag="lb")
            nc.scalar.activation(LB[:], EB[:], ACT.Ln, bias=1.0)
            OMT = spap.tile([P, 2], F32, tag="omt")
            nc.vector.tensor_scalar(OMT[:], TX[:], -1.0, 1.0, ALU.mult,
                                    ALU.add)
            VV = spap.tile([P, 2], F32, tag="vv")
            nc.vector.tensor_mul(VV[:], OMT[:], CX[:])
            nc.vector.tensor_add(VV[:], VV[:], LB[:])
            SCR2 = spap.tile([P, 2], F32, tag="scr2")
            nc.vector.tensor_scalar(SCR2[:], VV[:], OSC[:], 0.0, ALU.mult,
                                    ALU.add,
                                    accum_out=SACC[:, 3 * li:3 * li + 1])
            # wh
            T1 = spap.tile([P, 2], F32, tag="t1")
            nc.vector.tensor_scalar(T1[:], Sp[:, 3:5], IMG_W, IV[:], ALU.mult,
                                    ALU.add)
            nc.scalar.activation(T1[:], T1[:], ACT.Ln)
            T2 = spap.tile([P, 2], F32, tag="t2")
            nc.vector.tensor_scalar_add(T2[:], Sp[:, 87:89], IV[:])
            nc.scalar.activation(T2[:], T2[:], ACT.Ln)
            nc.vector.tensor_sub(T1[:], T1[:], T2[:])   # true_wh
            EW2 = spap.tile([P, 2], F32, tag="ew2")
            nc.scalar.activation(EW2[:], Sp[:, 92:94], ACT.Exp)
            AN = spap.tile([P, 2], F32, tag="an")
            nc.vector.tensor_scalar_mul(AN[:], Sp[:, 87:89], 1.0 / W)
            nc.vector.tensor_mul(EW2[:], EW2[:], AN[:])  # pred wh
            nc.vector.tensor_sub(T1[:], T1[:], EW2[:])
            DW2 = spap.tile([P, 2], F32, tag="dw2")
            nc.scalar.activation(DW2[:], T1[:], ACT.Square)
            OSC5 = spap.tile([P, 1], F32, tag="osc5")
            nc.vector.tensor_scalar_mul(OSC5[:], OSC[:], 0.5)
            nc.vector.tensor_scalar(SCR2[:], DW2[:], OSC5[:], 0.0, ALU.mult,
                                    ALU.add,
                                    accum_out=SACC[:, 3 * li + 1:3 * li + 2])
            # cls
            EC = spap.tile([P, 80], F32, tag="ec")
            nc.scalar.activation(EC[:], Sp[:, 94:174], ACT.Exp, scale=-1.0)
            nc.vector.tensor_scalar_add(EC[:], EC[:], 1.0)
            SGC = spap.tile([P, 80], F32, tag="sgc")
            nc.vector.reciprocal(SGC[:], EC[:])
            EB2 = spap.tile([P, 80], F32, tag="eb2")
            nc.scalar.activation(EB2[:], SGC[:], ACT.Exp, scale=-1.0)
            LB2 = spap.tile([P, 80], F32, tag="lb2")
            nc.scalar.activation(LB2[:], EB2[:], ACT.Ln, bias=1.0)
            OM2 = spap.tile([P, 80], F32, tag="om2")
            nc.vector.tensor_scalar(OM2[:], Sp[:, 5:85], -1.0, 1.0, ALU.mult,
                                    ALU.add)
            nc.vector.tensor_mul(OM2[:], OM2[:], SGC[:])
            nc.vector.tensor_add(OM2[:], OM2[:], LB2[:])
            SCR3 = spap.tile([P, 80], F32, tag="scr3")
            nc.vector.tensor_scalar(SCR3[:], OM2[:], obj, 0.0, ALU.mult,
                                    ALU.add,
                                    accum_out=SACC[:, 3 * li + 2:3 * li + 3])

        # ---- final combine ----
        AC3 = ACCD[:].rearrange("p (x t) -> p x t", t=3)
        TMP = accp.tile([P, B_CORE * 3], F32)
        nc.vector.tensor_add(TMP[:], AC3[:, :, 0], AC3[:, :, 1])
        nc.vector.tensor_tensor(TMP[:], TMP[:], AC3[:, :, 2], ALU.subtract)
        FIN = accp.tile([P, B_CORE], F32)
        nc.vector.tensor_reduce(
            FIN[:], TMP[:].rearrange("p (i l) -> p i l", l=3),
            axis=AX.X, op=ALU.add)
        FSP = accp.tile([P, 1], F32)
        nc.vector.tensor_reduce(FSP[:], SACC[:], axis=AX.X, op=ALU.add)
        PL = pso.tile([B_CORE, 1], F32, tag="pl")
        nc.tensor.matmul(PL[:], FIN[:], ON128[:], start=True, stop=False)
        nc.tensor.matmul(PL[:], SELC[:], FSP[:], start=False, stop=True)
        OUT = accp.tile([B_CORE, 1], F32)
        nc.scalar.copy(OUT[:], PL[:])
        nc.sync.dma_start(out=loss_d[:], in_=OUT[:])

    nc.finalize()
    return nc


def _prep_core_inputs(y_true, pred_13, pred_26, pred_52):
    consts, percell = _make_consts()
    yt85 = np.asarray(y_true).reshape(32, 10647, 85)
    yt = np.empty((32, 10647, YW), np.float16)
    yt[:, :, 0:85] = yt85
    yt[:, :, 85:89] = percell[None]
    ps32 = [np.asarray(p).reshape(32, -1, 85)
            for p in (pred_13, pred_26, pred_52)]
    ps = [np.ascontiguousarray(p.astype(np.float16)) for p in ps32]
    pf = np.ascontiguousarray(
        np.concatenate([p[:, :, 0:5] for p in ps32], axis=1))
    in_maps = []
    for c in range(N_CORES):
        sl = slice(c * B_CORE, (c + 1) * B_CORE)
        m = {"yt": yt[sl], "p0": ps[0][sl], "p1": ps[1][sl],
             "p2": ps[2][sl], "pf": pf[sl]}
        m.update(consts)
        in_maps.append(m)
    return in_maps


def kernel(y_true, pred_13, pred_26, pred_52):
    from concourse.bass_utils import run_bass_kernel_spmd

    Ms = [8, 16, 28]
    obj = np.asarray(y_true)[..., 0].reshape(32, 10647)
    cnt = [obj[:, LAYERS[i]["coff"]:LAYERS[i]["coff"] + LAYERS[i]["N"]]
           .sum(1).max() for i in range(3)]
    if any(cnt[i] > Ms[i] for i in range(3)):
        Ms = [64, 64, 64]
    key = tuple(Ms)
    if key not in _NC_CACHE:
        _NC_CACHE[key] = build_nc(Ms)
    nc = _NC_CACHE[key]

    in_maps = _prep_core_inputs(y_true, pred_13, pred_26, pred_52)
    res = run_bass_kernel_spmd(nc, in_maps, core_ids=list(range(N_CORES)))
    out = np.concatenate([r["loss"].reshape(B_CORE) for r in res.results])
    return out.astype(np.float32)


# revision 30
# speedup vs baseline: 1.1792x; 1.0316x over previous
"""YOLO loss (nms_detection) Trainium2 Bass kernel.

Data parallel over 8 NeuronCores (4 images per core). Per (image, layer):
  - y_true is host-augmented with per-cell (gx, gy, aw, ah) -> 89 channels,
    so one TensorEngine gather fetches labels + grid + anchors together.
  - inputs are host-cast to fp16 and cells quad-packed per partition row
    so every DMA descriptor moves >= 680B contiguous at half the bytes.
  - decode pred boxes (sigmoid via exp+reciprocal; one ACT table set).
  - obj compaction: row cumsum (tensor_tensor_scan) + triangular-matmul
    partition offsets -> rank; one-hot S = (iota == rank*obj).
  - gather true-box rows via fp16 matmuls (256-wide two-block rhs);
    dense decode reads a small fp32 copy of the conf/xy/wh channels.
  - broadcast box quantities via DRAM-roundtrip DMA.
  - IoU ignore mask in fp16: big [128, S, M] broadcast-AP DVE ops testing
    3*inter >= a1+a2 (equiv. IoU >= 0.5, no division).
  - dense conf BCE on c = sigmoid(x) with weight max(obj, ignore)*valid;
    obj-masked xy/wh/cls losses on the gathered [M, 174] rows only.
"""

from contextlib import ExitStack

import numpy as np

ANCHORS = np.array([[116., 90.], [156., 198.], [373., 326.],
                    [30., 61.], [62., 45.], [59., 119.],
                    [10., 13.], [16., 30.], [33., 23.]], dtype=np.float32)
IMG_W = 416.0
P = 128
B_CORE = 4
N_CORES = 8
YW = 89           # augmented y_true row: 85 + (gx, gy, aw, ah)
PW = 85
RW = 4 * YW + 4 * PW   # 696: [ytA..ytD | predA..predD] (fp16, quad cells)
TAILPAD = 48      # gather rhs reads up to row_base + 696 + 43 -> pad 48
SW = 174          # gathered sparse row: yt_aug 89 + pred 85
PADV = -60.0

# per-layer: N cells, slots S (=ceil(N/128) padded even), grid W, offsets
LAYERS = [
    dict(N=507,  S=4,  W=13.0, coff=0,    goff=0),
    dict(N=2028, S=16, W=26.0, coff=507,  goff=4),
    dict(N=8112, S=64, W=52.0, coff=2535, goff=20),
]
STOT = 84

_NC_CACHE = {}


def _make_consts():
    # dense grid/anchor const: (gxw, gyw, awhalf, ahhalf, valid)
    gad = np.zeros((P, STOT, 5), np.float32)
    # per-cell ga columns appended to y_true, in flat cell order
    percell = np.zeros((10647, 4), np.float32)
    for li, lay in enumerate(LAYERS):
        W = int(lay["W"])
        N, S, goff, coff = lay["N"], lay["S"], lay["goff"], lay["coff"]
        c = np.arange(N)
        percell[coff:coff + N, 0] = (c % (W * 3)) // 3
        percell[coff:coff + N, 1] = c // (W * 3)
        percell[coff:coff + N, 2] = ANCHORS[3 * li + (c % 3), 0]
        percell[coff:coff + N, 3] = ANCHORS[3 * li + (c % 3), 1]
        p = np.arange(P)[:, None]
        s = np.arange(S)[None, :]
        cell = (s // 4) * 512 + 4 * p + (s % 4)
        valid = cell < N
        cc = np.minimum(cell, N - 1)
        aw = ANCHORS[3 * li + (cc % 3), 0]
        ah = ANCHORS[3 * li + (cc % 3), 1]
        gx = ((cc % (W * 3)) // 3).astype(np.float32)
        gy = (cc // (W * 3)).astype(np.float32)
        gad[:, goff:goff + S, 0] = np.where(valid, gx / W, 0)
        gad[:, goff:goff + S, 1] = np.where(valid, gy / W, 0)
        gad[:, goff:goff + S, 2] = np.where(valid, aw / (2.0 * W), 0)
        gad[:, goff:goff + S, 3] = np.where(valid, ah / (2.0 * W), 0)
        gad[:, goff:goff + S, 4] = valid.astype(np.float32)
    ut = np.triu(np.ones((P, P), np.float32), 1)  # ut[q,p]=1 iff q<p
    ones128 = np.ones((P, 1), np.float32)
    sel = np.zeros((P, B_CORE), np.float32)
    for i in range(B_CORE):
        sel[32 * i:32 * (i + 1), i] = 1.0
    return {"gad": gad, "ut": ut, "ones128": ones128, "sel": sel}, percell


def build_nc(Ms):
    import concourse.bass as bass
    import concourse.bacc as bacc
    import concourse.mybir as mybir
    from concourse.tile import TileContext

    F32 = mybir.dt.float32
    F16 = mybir.dt.float16
    F32R = mybir.dt.float32r
    ALU = mybir.AluOpType
    ACT = mybir.ActivationFunctionType
    AX = mybir.AxisListType
    MM = max(Ms)

    nc = bacc.Bacc()
    yt_d = nc.dram_tensor("yt", [B_CORE, 10647, YW], F16,
                          kind="ExternalInput")
    pr_d = [nc.dram_tensor(f"p{i}", [B_CORE, LAYERS[i]["N"], PW], F16,
                           kind="ExternalInput") for i in range(3)]
    pf_d = nc.dram_tensor("pf", [B_CORE, 10647, 5], F32,
                          kind="ExternalInput")
    ga_d = nc.dram_tensor("gad", [P, STOT, 5], F32, kind="ExternalInput")
    ut_d = nc.dram_tensor("ut", [P, P], F32, kind="ExternalInput")
    on_d = nc.dram_tensor("ones128", [P, 1], F32, kind="ExternalInput")
    se_d = nc.dram_tensor("sel", [P, B_CORE], F32, kind="ExternalInput")
    loss_d = nc.dram_tensor("loss", [B_CORE, 1], F32, kind="ExternalOutput")

    def bmid(ap2, n):
        # [P, X] -> [P, n, X] (step-0 middle dim)
        return bass.AP(tensor=ap2.tensor, offset=ap2.offset,
                       ap=[ap2.ap[0], [0, n]] + ap2.ap[1:])

    big = MM > 32   # fallback config must fit SBUF with M=64
    with TileContext(nc) as tc, ExitStack() as ctx:
        cpool = ctx.enter_context(tc.tile_pool(name="consts", bufs=1))
        combp = {li: ctx.enter_context(
            tc.tile_pool(name=f"comb{li}",
                         bufs=1 if (big and li == 2) else 3))
                 for li in range(3)}
        decp = ctx.enter_context(tc.tile_pool(name="dec", bufs=2 if big else 4))
        ioup = ctx.enter_context(tc.tile_pool(name="iou", bufs=1 if big else 2))
        stp = ctx.enter_context(tc.tile_pool(name="st", bufs=2 if big else 4))
        gatp = ctx.enter_context(
            tc.tile_pool(name="gat", bufs=2 if big else 3))
        spap = ctx.enter_context(tc.tile_pool(name="spa", bufs=1))
        accp = ctx.enter_context(tc.tile_pool(name="acc", bufs=1))
        drp = ctx.enter_context(
            tc.tile_pool(name="scr", bufs=3, space=bass.MemorySpace.DRAM))
        psg = ctx.enter_context(
            tc.tile_pool(name="psg", bufs=3, space=bass.MemorySpace.PSUM))
        pso = ctx.enter_context(
            tc.tile_pool(name="pso", bufs=2, space=bass.MemorySpace.PSUM))

        GAD = cpool.tile([P, STOT, 5], F32)
        nc.sync.dma_start(out=GAD, in_=ga_d[:])
        UT = cpool.tile([P, P], F32)
        nc.sync.dma_start(out=UT, in_=ut_d[:])
        ON128 = cpool.tile([P, 1], F32)
        nc.sync.dma_start(out=ON128, in_=on_d[:])
        SELC = cpool.tile([P, B_CORE], F32)
        nc.sync.dma_start(out=SELC, in_=se_d[:])
        IOTA = cpool.tile([P, MM], F32)
        nc.gpsimd.iota(IOTA[:], [[1, MM]], base=1, channel_multiplier=0,
                       allow_small_or_imprecise_dtypes=True)
        ZER = cpool.tile([P, 64], F32)
        nc.gpsimd.memset(ZER[:], 0.0)

        ACCD = accp.tile([P, B_CORE * 9], F32)   # (img, layer, term) dense
        SACC = accp.tile([P, 9], F32)            # (layer, term) sparse
        nc.gpsimd.memset(SACC[:], 0.0)
        SPA = {li: spap.tile([P, SW], F32, tag=f"spa{li}", name=f"spa{li}")
               for li in range(3)}
        for li in range(3):
            nc.gpsimd.memset(SPA[li][:], 0.0)

        for img, li in [(i, l) for l in (2, 1, 0) for i in range(B_CORE)]:
            if True:
                lay = LAYERS[li]
                N, S, W, coff, goff = (lay["N"], lay["S"], lay["W"],
                                       lay["coff"], lay["goff"])
                M = Ms[li]
                Gp = S // 4                # quad rows
                full = N // 512            # full quad rows
                remc = N - full * 512
                rem_p = remc // 4
                odd = remc % 4             # 0..3 extra cells on one partition
                CF = combp[li].tile([P, Gp * RW + TAILPAD], F16,
                                    tag=f"comb{li}", name=f"comb{li}_{img}")
                cfl = CF[:]
                pstride = cfl.ap[0]

                def yv(c0, c1, _a=cfl, _g=Gp):
                    # yt view [P, Gp, 4, c1-c0]
                    return bass.AP(tensor=_a.tensor, offset=_a.offset + c0,
                                   ap=[_a.ap[0], [RW, _g], [YW, 4],
                                       [1, c1 - c0]])

                def pv(c0, c1, _a=cfl, _g=Gp):
                    return bass.AP(tensor=_a.tensor,
                                   offset=_a.offset + 4 * YW + c0,
                                   ap=[_a.ap[0], [RW, _g], [PW, 4],
                                       [1, c1 - c0]])

                def cview(off, n, _a=cfl):
                    return bass.AP(tensor=_a.tensor, offset=_a.offset + off,
                                   ap=[_a.ap[0], [1, n]])

                # pad init: tail cols + last quad row (dma overwrites live)
                nc.vector.memset(cview(Gp * RW, TAILPAD), 0.0)
                if remc:
                    nc.vector.memset(cview((Gp - 1) * RW, 4 * YW), 0.0)
                    nc.vector.memset(cview((Gp - 1) * RW + 4 * YW, 4 * PW),
                                     PADV)
                # ---- loads (contiguous >= 680B elements) ----
                ysrc = yt_d[img]
                ybase = ysrc.offset + coff * YW
                if full:
                    nc.sync.dma_start(
                        out=bass.AP(tensor=cfl.tensor, offset=cfl.offset,
                                    ap=[[pstride[0], P], [RW, full],
                                        [1, 4 * YW]]),
                        in_=bass.AP(tensor=ysrc.tensor, offset=ybase,
                                    ap=[[4 * YW, P], [512 * YW, full],
                                        [1, 4 * YW]]))
                if rem_p:
                    nc.sync.dma_start(
                        out=bass.AP(tensor=cfl.tensor,
                                    offset=cfl.offset + full * RW,
                                    ap=[[pstride[0], rem_p], [1, 4 * YW]]),
                        in_=bass.AP(tensor=ysrc.tensor,
                                    offset=ybase + full * 512 * YW,
                                    ap=[[4 * YW, rem_p], [1, 4 * YW]]))
                if odd:
                    nc.sync.dma_start(
                        out=CF[rem_p:rem_p + 1,
                               full * RW:full * RW + odd * YW],
                        in_=bass.AP(
                            tensor=ysrc.tensor,
                            offset=ybase + (full * 512 + 4 * rem_p) * YW,
                            ap=[[odd * YW, 1], [1, odd * YW]]))
                psrc = pr_d[li][img]
                pbase = psrc.offset
                if full:
                    nc.sync.dma_start(
                        out=bass.AP(tensor=cfl.tensor,
                                    offset=cfl.offset + 4 * YW,
                                    ap=[[pstride[0], P], [RW, full],
                                        [1, 4 * PW]]),
                        in_=bass.AP(tensor=psrc.tensor, offset=pbase,
                                    ap=[[4 * PW, P], [512 * PW, full],
                                        [1, 4 * PW]]))
                if rem_p:
                    nc.sync.dma_start(
                        out=bass.AP(tensor=cfl.tensor,
                                    offset=cfl.offset + full * RW + 4 * YW,
                                    ap=[[pstride[0], rem_p], [1, 4 * PW]]),
                        in_=bass.AP(tensor=psrc.tensor,
                                    offset=pbase + full * 512 * PW,
                                    ap=[[4 * PW, rem_p], [1, 4 * PW]]))
                if odd:
                    nc.sync.dma_start(
                        out=CF[rem_p:rem_p + 1,
                               full * RW + 4 * YW:full * RW + 4 * YW +
                               odd * PW],
                        in_=bass.AP(
                            tensor=psrc.tensor,
                            offset=pbase + (full * 512 + 4 * rem_p) * PW,
                            ap=[[odd * PW, 1], [1, odd * PW]]))

                # fp32 front pred channels (conf, xy, wh) for dense decode
                FW = 20  # 4 cells x 5 ch
                PF = decp.tile([P, Gp * FW + 20], F32, tag="pf",
                               name=f"pf{li}_{img}")
                pfl = PF[:]
                pfsrc = pf_d[img]
                pfbase = pfsrc.offset + coff * 5
                if remc:
                    nc.vector.memset(
                        bass.AP(tensor=pfl.tensor,
                                offset=pfl.offset + (Gp - 1) * FW,
                                ap=[pfl.ap[0], [1, FW]]), PADV)
                nc.vector.memset(
                    bass.AP(tensor=pfl.tensor, offset=pfl.offset + Gp * FW,
                            ap=[pfl.ap[0], [1, 20]]), PADV)
                if full:
                    nc.sync.dma_start(
                        out=bass.AP(tensor=pfl.tensor, offset=pfl.offset,
                                    ap=[[pfl.ap[0][0], P], [FW, full],
                                        [1, FW]]),
                        in_=bass.AP(tensor=pfsrc.tensor, offset=pfbase,
                                    ap=[[FW, P], [512 * 5, full], [1, FW]]))
                if rem_p:
                    nc.sync.dma_start(
                        out=bass.AP(tensor=pfl.tensor,
                                    offset=pfl.offset + full * FW,
                                    ap=[[pfl.ap[0][0], rem_p], [1, FW]]),
                        in_=bass.AP(tensor=pfsrc.tensor,
                                    offset=pfbase + full * 512 * 5,
                                    ap=[[FW, rem_p], [1, FW]]))
                if odd:
                    nc.sync.dma_start(
                        out=PF[rem_p:rem_p + 1,
                               full * FW:full * FW + odd * 5],
                        in_=bass.AP(
                            tensor=pfsrc.tensor,
                            offset=pfbase + (full * 512 + 4 * rem_p) * 5,
                            ap=[[odd * 5, 1], [1, odd * 5]]))

                def pfv(c0, c1, _a=pfl, _g=Gp):
                    return bass.AP(tensor=_a.tensor, offset=_a.offset + c0,
                                   ap=[_a.ap[0], [FW, _g], [5, 4],
                                       [1, c1 - c0]])

                # compact copies of the interleaved dense channels
                OBJC = decp.tile([P, S], F32, tag="objc")
                oc = OBJC[:]
                nc.vector.tensor_copy(
                    bass.AP(tensor=oc.tensor, offset=oc.offset,
                            ap=[oc.ap[0], [4, Gp], [1, 4]]),
                    yv(0, 1).squeeze(3))
                XCF = decp.tile([P, S], F32, tag="xcf")
                xc = XCF[:]
                nc.scalar.copy(
                    bass.AP(tensor=xc.tensor, offset=xc.offset,
                            ap=[xc.ap[0], [4, Gp], [1, 4]]),
                    pfv(0, 1).squeeze(3))

                def compact2(tile):   # [P, Gp, 4, 2] view over [P, S, 2]
                    a = tile[:]
                    return bass.AP(tensor=a.tensor, offset=a.offset,
                                   ap=[a.ap[0], [8, Gp], [2, 4], [1, 2]])

                # ---- decode dense ----
                EXY = decp.tile([P, S, 2], F32, tag="exy")
                nc.scalar.activation(compact2(EXY), pfv(1, 3), ACT.Exp,
                                     scale=-1.0)
                nc.vector.tensor_scalar_add(EXY[:], EXY[:], 1.0)
                SGX = decp.tile([P, S, 2], F32, tag="sgx")
                nc.vector.reciprocal(SGX[:], EXY[:])
                CXY = decp.tile([P, S, 2], F32, tag="cxy")
                nc.vector.scalar_tensor_tensor(
                    CXY[:], SGX[:], 1.0 / W, GAD[:, goff:goff + S, 0:2],
                    ALU.mult, ALU.add)
                EWH = decp.tile([P, S, 2], F32, tag="ewh")
                nc.scalar.activation(compact2(EWH), pfv(3, 5), ACT.Exp)
                HWT = decp.tile([P, S, 2], F32, tag="hwt")
                nc.vector.tensor_mul(HWT[:], EWH[:],
                                     GAD[:, goff:goff + S, 2:4])
                PMX = decp.tile([P, S, 2], F16, tag="pmx")
                nc.vector.tensor_add(PMX[:], CXY[:], HWT[:])
                PMN = decp.tile([P, S, 2], F16, tag="pmn")
                nc.vector.tensor_sub(PMN[:], CXY[:], HWT[:])
                A13 = decp.tile([P, S], F16, tag="a13")
                nc.vector.scalar_tensor_tensor(
                    A13[:], HWT[:, :, 0], 4.0 / 3.0, HWT[:, :, 1],
                    ALU.mult, ALU.mult)

                # ---- rank & one-hot selection ----
                RCUM = decp.tile([P, S], F32, tag="rcum")
                nc.vector.tensor_tensor_scan(RCUM[:], OBJC[:], ZER[:, 0:S],
                                             0.0, ALU.add, ALU.add)
                OFFP = pso.tile([P, 1], F32, tag="offp")
                nc.tensor.matmul(OFFP[:], UT[:], RCUM[:, S - 1:S],
                                 start=True, stop=True)
                RANK = decp.tile([P, S], F32, tag="rank")
                nc.vector.tensor_scalar_add(RANK[:], RCUM[:], OFFP[:])
                RPM = decp.tile([P, S], F32, tag="rpm")
                nc.vector.tensor_mul(RPM[:], RANK[:], OBJC[:])
                STT = stp.tile([P, S, M], F16, tag="st")
                nc.vector.tensor_tensor(STT[:], bmid(IOTA[:, 0:M], S),
                                        RPM[:].broadcast_to([P, S, M]),
                                        ALU.is_equal)

                # ---- gather true rows (PE, fp16, 256-wide 2-block rhs) ----
                PGA = psg.tile([MM, 256], F32, tag="pga")
                for s in range(S):
                    g, j = s // 4, s % 4
                    yoff = g * RW + j * YW
                    delta = 4 * YW + j * PW - j * YW  # 356 - 4*j
                    rhs = bass.AP(tensor=cfl.tensor,
                                  offset=cfl.offset + yoff,
                                  ap=[[pstride[0], P], [delta, 2], [1, 128]])
                    nc.tensor.matmul(PGA[0:M, :], STT[:, s, :],
                                     rhs, start=(s == 0), stop=(s == S - 1))
                SPT = gatp.tile([MM, SW], F32, tag="spt")
                nc.scalar.copy(SPT[0:M, 0:YW], PGA[0:M, 0:YW])
                nc.scalar.copy(SPT[0:M, YW:SW], PGA[0:M, 128:128 + PW])
                nc.sync.dma_start(out=SPA[li][32 * img:32 * img + M, :],
                                  in_=SPT[0:M, :])
                # box rows (obj,x,y,w,h) -> dram -> [P,5,M] broadcast
                SCR = drp.tile([5, MM], F32, tag="scr")
                s1 = SPT[0:M, 0:5]
                s1t = bass.AP(tensor=s1.tensor, offset=s1.offset,
                              ap=[s1.ap[0], [1, 5], [1, 1]])
                d1 = bass.AP(tensor=SCR[:].tensor, offset=SCR[:].offset,
                             ap=[[1, M], [MM, 5], [1, 1]])
                nc.sync.dma_start(out=d1, in_=s1t)
                RAWB = gatp.tile([P, 5, MM], F32, tag="rawb")
                s2 = bass.AP(tensor=SCR[:].tensor, offset=SCR[:].offset,
                             ap=[[0, P], [1, 5 * MM]])
                d2 = bass.AP(tensor=RAWB[:].tensor, offset=RAWB[:].offset,
                             ap=[RAWB[:].ap[0], [1, 5 * MM]])
                nc.sync.dma_start(out=d2, in_=s2)

                BT = gatp.tile([P, 5, MM], F16, tag="bt")
                X_, Y_, W_, H_ = (RAWB[:, 1, 0:M], RAWB[:, 2, 0:M],
                                  RAWB[:, 3, 0:M], RAWB[:, 4, 0:M])
                nc.vector.scalar_tensor_tensor(BT[:, 0, 0:M], W_, -0.5, X_,
                                               ALU.mult, ALU.add)
                nc.vector.scalar_tensor_tensor(BT[:, 1, 0:M], W_, 0.5, X_,
                                               ALU.mult, ALU.add)
                nc.vector.scalar_tensor_tensor(BT[:, 2, 0:M], H_, -0.5, Y_,
                                               ALU.mult, ALU.add)
                nc.vector.scalar_tensor_tensor(BT[:, 3, 0:M], H_, 0.5, Y_,
                                               ALU.mult, ALU.add)
                nc.vector.scalar_tensor_tensor(BT[:, 4, 0:M], W_, 1.0 / 3.0,
                                               H_, ALU.mult, ALU.mult)

                # ---- IoU ignore: smax = max_m(rx*ry - a1/3 - a2/3) ----
                shp = [P, S, M]
                IX = ioup.tile(shp, F16, tag="ix")
                nc.vector.tensor_tensor(IX[:], PMX[:, :, 0].broadcast_to(shp),
                                        bmid(BT[:, 1, 0:M], S), ALU.min)
                JX = ioup.tile(shp, F16, tag="jx")
                nc.vector.tensor_tensor(JX[:], PMN[:, :, 0].broadcast_to(shp),
                                        bmid(BT[:, 0, 0:M], S), ALU.max)
                nc.vector.tensor_sub(IX[:], IX[:], JX[:])
                nc.scalar.activation(IX[:], IX[:], ACT.Relu)
                IY = ioup.tile(shp, F16, tag="iy")
                nc.vector.tensor_tensor(IY[:], PMX[:, :, 1].broadcast_to(shp),
                                        bmid(BT[:, 3, 0:M], S), ALU.min)
                JY = ioup.tile(shp, F16, tag="jy")
                nc.vector.tensor_tensor(JY[:], PMN[:, :, 1].broadcast_to(shp),
                                        bmid(BT[:, 2, 0:M], S), ALU.max)
                nc.vector.tensor_sub(IY[:], IY[:], JY[:])
                nc.scalar.activation(IY[:], IY[:], ACT.Relu)
                nc.vector.tensor_mul(JX[:], IX[:], IY[:])
                nc.vector.tensor_tensor(JX[:], JX[:],
                                        A13[:].broadcast_to(shp),
                                        ALU.subtract)
                nc.vector.tensor_tensor(JX[:], JX[:], bmid(BT[:, 4, 0:M], S),
                                        ALU.subtract)
                SMX = decp.tile([P, S], F32, tag="smx")
                nc.vector.tensor_reduce(SMX[:], JX[:], axis=AX.X, op=ALU.max)

                # ---- dense conf loss (on c = sigmoid(x)) ----
                WT = decp.tile([P, S], F32, tag="wt")
                nc.vector.scalar_tensor_tensor(WT[:], SMX[:], 0.0, OBJC[:],
                                               ALU.is_lt, ALU.max)
                nc.vector.tensor_mul(WT[:], WT[:], GAD[:, goff:goff + S, 4])
                ECF = decp.tile([P, S], F32, tag="ecf")
                nc.scalar.activation(ECF[:], XCF[:], ACT.Exp, scale=-1.0)
                nc.vector.tensor_scalar_add(ECF[:], ECF[:], 1.0)
                CCF = decp.tile([P, S], F32, tag="ccf")
                nc.vector.reciprocal(CCF[:], ECF[:])
                E3T = decp.tile([P, S], F32, tag="e3t")
                nc.scalar.activation(E3T[:], CCF[:], ACT.Exp, scale=-1.0)
                L1T = decp.tile([P, S], F32, tag="l1t")
                nc.scalar.activation(L1T[:], E3T[:], ACT.Ln, bias=1.0)
                SCRP = decp.tile([P, S], F32, tag="scrp")
                base = img * 9 + li * 3
                nc.vector.scalar_tensor_tensor(
                    SCRP[:], CCF[:], 1.0, WT[:], ALU.mult, ALU.mult,
                    accum_out=ACCD[:, base:base + 1])
                nc.vector.scalar_tensor_tensor(
                    SCRP[:], L1T[:], 1.0, WT[:], ALU.mult, ALU.mult,
                    accum_out=ACCD[:, base + 1:base + 2])
                nc.vector.scalar_tensor_tensor(
                    SCRP[:], CCF[:], 1.0, OBJC[:], ALU.mult, ALU.mult,
                    accum_out=ACCD[:, base + 2:base + 3])

        # ---- sparse losses per layer (4 images batched on partitions) ----
        for li, lay in enumerate(LAYERS):
            W = lay["W"]
            Sp = SPA[li]
            obj = Sp[:, 0:1]
            WH1 = spap.tile([P, 1], F32, tag="wh1")
            nc.vector.tensor_mul(WH1[:], Sp[:, 3:4], Sp[:, 4:5])
            SC = spap.tile([P, 1], F32, tag="sc")
            nc.vector.tensor_scalar(SC[:], WH1[:], -1.0, 2.0, ALU.mult,
                                    ALU.add)
            OSC = spap.tile([P, 1], F32, tag="osc")
            nc.vector.tensor_mul(OSC[:], SC[:], obj)
            IV = spap.tile([P, 1], F32, tag="iv")
            nc.vector.tensor_scalar(IV[:], obj, -1.0, 1.0, ALU.mult, ALU.add)
            # xy
            EX = spap.tile([P, 2], F32, tag="ex")
            nc.scalar.activation(EX[:], Sp[:, 90:92], ACT.Exp, scale=-1.0)
            nc.vector.tensor_scalar_add(EX[:], EX[:], 1.0)
            SG = spap.tile([P, 2], F32, tag="sg")
            nc.vector.reciprocal(SG[:], EX[:])
            CX = spap.tile([P, 2], F32, tag="cx")
            nc.vector.tensor_add(CX[:], SG[:], Sp[:, 85:87])
            nc.vector.tensor_scalar_mul(CX[:], CX[:], 1.0 / W)
            TX = spap.tile([P, 2], F32, tag="tx")
            nc.vector.scalar_tensor_tensor(TX[:], Sp[:, 1:3], W, Sp[:, 85:87],
                                           ALU.mult, ALU.subtract)
            EB = spap.tile([P, 2], F32, tag="eb")
            nc.scalar.activation(EB[:], CX[:], ACT.Exp, scale=-1.0)
            LB = spap.tile([P, 2], F32, tag="lb")
            nc.scalar.activation(LB[:], EB[:], ACT.Ln, bias=1.0)
            OMT = spap.tile([P, 2], F32, tag="omt")
            nc.vector.tensor_scalar(OMT[:], TX[:], -1.0, 1.0, ALU.mult,
                                    ALU.add)
            VV = spap.tile([P, 2], F32, tag="vv")
            nc.vector.tensor_mul(VV[:], OMT[:], CX[:])
            nc.vector.tensor_add(VV[:], VV[:], LB[:])
            SCR2 = spap.tile([P, 2], F32, tag="scr2")
            nc.vector.tensor_scalar(SCR2[:], VV[:], OSC[:], 0.0, ALU.mult,
                                    ALU.add,
                                    accum_out=SACC[:, 3 * li:3 * li + 1])
            # wh
            T1 = spap.tile([P, 2], F32, tag="t1")
            nc.vector.tensor_scalar(T1[:], Sp[:, 3:5], IMG_W, IV[:], ALU.mult,
                                    ALU.add)
            nc.scalar.activation(T1[:], T1[:], ACT.Ln)
            T2 = spap.tile([P, 2], F32, tag="t2")
            nc.vector.tensor_scalar_add(T2[:], Sp[:, 87:89], IV[:])
            nc.scalar.activation(T2[:], T2[:], ACT.Ln)
            nc.vector.tensor_sub(T1[:], T1[:], T2[:])   # true_wh
            EW2 = spap.tile([P, 2], F32, tag="ew2")
            nc.scalar.activation(EW2[:], Sp[:, 92:94], ACT.Exp)
            AN = spap.tile([P, 2], F32, tag="an")
            nc.vector.tensor_scalar_mul(AN[:], Sp[:, 87:89], 1.0 / W)
            nc.vector.tensor_mul(EW2[:], EW2[:], AN[:])  # pred wh
            nc.vector.tensor_sub(T1[:], T1[:], EW2[:])
            DW2 = spap.tile([P, 2], F32, tag="dw2")
            nc.scalar.activation(DW2[:], T1[:], ACT.Square)
            OSC5 = spap.tile([P, 1], F32, tag="osc5")
            nc.vector.tensor_scalar_mul(OSC5[:], OSC[:], 0.5)
            nc.vector.tensor_scalar(SCR2[:], DW2[:], OSC5[:], 0.0, ALU.mult,
                                    ALU.add,
                                    accum_out=SACC[:, 3 * li + 1:3 * li + 2])
            # cls
            EC = spap.tile([P, 80], F32, tag="ec")
            nc.scalar.activation(EC[:], Sp[:, 94:174], ACT.Exp, scale=-1.0)
            nc.vector.tensor_scalar_add(EC[:], EC[:], 1.0)
            SGC = spap.tile([P, 80], F32, tag="sgc")
            nc.vector.reciprocal(SGC[:], EC[:])
            EB2 = spap.tile([P, 80], F32, tag="eb2")
            nc.scalar.activation(EB2[:], SGC[:], ACT.Exp, scale=-1.0)
            LB2 = spap.tile([P, 80], F32, tag="lb2")
            nc.scalar.activation(LB2[:], EB2[:], ACT.Ln, bias=1.0)
            OM2 = spap.tile([P, 80], F32, tag="om2")
            nc.vector.tensor_scalar(OM2[:], Sp[:, 5:85], -1.0, 1.0, ALU.mult,
                                    ALU.add)
            nc.vector.tensor_mul(OM2[:], OM2[:], SGC[:])
            nc.vector.tensor_add(OM2[:], OM2[:], LB2[:])
            SCR3 = spap.tile([P, 80], F32, tag="scr3")
            nc.vector.tensor_scalar(SCR3[:], OM2[:], obj, 0.0, ALU.mult,
                                    ALU.add,
                                    accum_out=SACC[:, 3 * li + 2:3 * li + 3])

        # ---- final combine ----
        AC3 = ACCD[:].rearrange("p (x t) -> p x t", t=3)
        TMP = accp.tile([P, B_CORE * 3], F32)
        nc.vector.tensor_add(TMP[:], AC3[:, :, 0], AC3[:, :, 1])
        nc.vector.tensor_tensor(TMP[:], TMP[:], AC3[:, :, 2], ALU.subtract)
        FIN = accp.tile([P, B_CORE], F32)
        nc.vector.tensor_reduce(
            FIN[:], TMP[:].rearrange("p (i l) -> p i l", l=3),
            axis=AX.X, op=ALU.add)
        FSP = accp.tile([P, 1], F32)
        nc.vector.tensor_reduce(FSP[:], SACC[:], axis=AX.X, op=ALU.add)
        PL = pso.tile([B_CORE, 1], F32, tag="pl")
        nc.tensor.matmul(PL[:], FIN[:], ON128[:], start=True, stop=False)
        nc.tensor.matmul(PL[:], SELC[:], FSP[:], start=False, stop=True)
        OUT = accp.tile([B_CORE, 1], F32)
        nc.scalar.copy(OUT[:], PL[:])
        nc.sync.dma_start(out=loss_d[:], in_=OUT[:])

    nc.finalize()
    return nc


def _prep_core_inputs(y_true, pred_13, pred_26, pred_52):
    consts, percell = _make_consts()
    yt85 = np.asarray(y_true).reshape(32, 10647, 85)
    yt = np.empty((32, 10647, YW), np.float16)
    yt[:, :, 0:85] = yt85
    yt[:, :, 85:89] = percell[None]
    ps32 = [np.asarray(p).reshape(32, -1, 85)
            for p in (pred_13, pred_26, pred_52)]
    ps = [np.ascontiguousarray(p.astype(np.float16)) for p in ps32]
    pf = np.ascontiguousarray(
        np.concatenate([p[:, :, 0:5] for p in ps32], axis=1))
    in_maps = []
    for c in range(N_CORES):
        sl = slice(c * B_CORE, (c + 1) * B_CORE)
        m = {"yt": yt[sl], "p0": ps[0][sl], "p1": ps[1][sl],
             "p2": ps[2][sl], "pf": pf[sl]}
        m.update(consts)
        in_maps.append(m)
    return in_maps


def kernel(y_true, pred_13, pred_26, pred_52):
    from concourse.bass_utils import run_bass_kernel_spmd

    Ms = [8, 16, 28]
    obj = np.asarray(y_true)[..., 0].reshape(32, 10647)
    cnt = [obj[:, LAYERS[i]["coff"]:LAYERS[i]["coff"] + LAYERS[i]["N"]]
           .sum(1).max() for i in range(3)]
    if any(cnt[i] > Ms[i] for i in range(3)):
        Ms = [64, 64, 64]
    key = tuple(Ms)
    if key not in _NC_CACHE:
        _NC_CACHE[key] = build_nc(Ms)
    nc = _NC_CACHE[key]

    in_maps = _prep_core_inputs(y_true, pred_13, pred_26, pred_52)
    res = run_bass_kernel_spmd(nc, in_maps, core_ids=list(range(N_CORES)))
    out = np.concatenate([r["loss"].reshape(B_CORE) for r in res.results])
    return out.astype(np.float32)
